# revision 3
# baseline (speedup 1.0000x reference)
"""MidGCN forward on 8 Trainium2 NeuronCores (Bass/Tile, SPMD row-sharding).

Math (alpha = 0.5):
  DAD   = d_row * adj * d_col          (d = rsqrt of row/col sums)
  adj_f = (0.5*I - DAD)(I + DAD) = 0.5*I - 0.5*DAD - DAD@DAD
  h     = relu(adj_f @ (x @ W1))
  out   = log_softmax(adj_f @ (h @ W2) + b2)

Key rewrite: never materialize adj_f / DAD@DAD.  For narrow v:
  adj_f @ v = 0.5*v - 0.5*(DAD@v) - DAD@(DAD@v)
and DAD@v = d_row * (adj @ (d_col * v)), so each application is an
adj @ (narrow) matmul with tiny diagonal scalings folded into the
narrow operands.

Sharding: core i holds rows_i = [1024*i, 1024*(i+1)) of adj, as the
host-transposed slab adjT_i = adj[rows_i, :].T  (shape [8192, 1024],
bf16) which stays resident in SBUF.  Column sums: local free-axis
reduce + ReduceScatter(add) (each core receives exactly its d_col
slice).  Row sums: free via a ones-column appended to the first pass'
rhs.  Narrow activations are scaled shard-wise and AllGathered (bf16)
between passes.  Weights are replicated.  Output: each core produces
its own [1024, 2] log-softmax rows; host concatenates.
"""

import numpy as np
import ml_dtypes

NCORE = 8
N = 8192
NF = 512
NH = 256
NC = 2
RPC = N // NCORE          # rows per core = 1024
KT = N // 128             # 64 contraction tiles
MT = RPC // 128           # 8 output row tiles per core
FT = NF // 128            # 4 k-tiles for x @ W1
ZW = NH + 2               # pass-1 rhs width: 256 cols + ones col + pad

_CACHE = {}


def _build():
    import concourse.bass as bass
    import concourse.mybir as mybir
    import concourse.tile as tile
    from concourse import bacc, masks
    from concourse.bass import ts

    BF = mybir.dt.bfloat16
    F32 = mybir.dt.float32
    AX = mybir.AxisListType
    OP = mybir.AluOpType
    AF = mybir.ActivationFunctionType

    nc = bacc.Bacc("TRN2", target_bir_lowering=False, debug=False,
                   num_devices=NCORE)

    adjT = nc.dram_tensor("adjT", [N, RPC], BF, kind="ExternalInput")
    xT = nc.dram_tensor("xT", [NF, RPC], BF, kind="ExternalInput")
    w1 = nc.dram_tensor("w1", [NF, NH], BF, kind="ExternalInput")
    w2h = nc.dram_tensor("w2h", [NH, NC], BF, kind="ExternalInput")
    b2 = nc.dram_tensor("b2", [1, NC], F32, kind="ExternalInput")
    out = nc.dram_tensor("out", [RPC, NC], F32, kind="ExternalOutput")

    cs_in = nc.dram_tensor("cs_in", [N], F32)
    cs_out = nc.dram_tensor("cs_out", [RPC], F32)
    zs_in = nc.dram_tensor("zs_in", [RPC, ZW], BF)
    zs_out = nc.dram_tensor("zs_out", [N, ZW], BF, addr_space="Shared")
    zt_in = nc.dram_tensor("zt_in", [RPC, NH], BF)
    zt_out = nc.dram_tensor("zt_out", [N, NH], BF, addr_space="Shared")
    zv_in = nc.dram_tensor("zv_in", [RPC, NC], BF)
    zv_out = nc.dram_tensor("zv_out", [N, NC], BF, addr_space="Shared")
    zu_in = nc.dram_tensor("zu_in", [RPC, NC], BF)
    zu_out = nc.dram_tensor("zu_out", [N, NC], BF, addr_space="Shared")
    RG = [list(range(NCORE))]

    with tile.TileContext(nc) as tc:
        from contextlib import ExitStack
        with ExitStack() as ctx:
            p_adj = ctx.enter_context(tc.tile_pool(name="p_adj", bufs=KT))
            p_zb = ctx.enter_context(tc.tile_pool(name="p_zb", bufs=KT))
            p_one = ctx.enter_context(tc.tile_pool(name="p_one", bufs=1))
            p_rot = ctx.enter_context(tc.tile_pool(name="p_rot", bufs=2))

            # ---------- persistent SBUF ----------
            csp = p_one.tile([128, KT], F32, tag="csp")
            s_sb = p_one.tile([128, MT, NH], F32, tag="s")
            xT_sb = p_one.tile([128, FT, RPC], BF, tag="xT")
            w1_sb = p_one.tile([128, FT, NH], BF, tag="w1")
            w2_sb = p_one.tile([128, NC, NC], BF, tag="w2")   # [256k, 2] as 2 tiles
            b2_sb = p_one.tile([128, NC], F32, tag="b2")
            ident = p_one.tile([128, 128], BF, tag="ident")
            cloc = p_one.tile([128, MT], F32, tag="cloc")
            dcol = p_one.tile([128, MT], F32, tag="dcol")
            drow = p_one.tile([128, MT], F32, tag="drow")
            evec = p_one.tile([128, MT], F32, tag="evec")
            n2dr = p_one.tile([128, MT], F32, tag="n2dr")
            ndr = p_one.tile([128, MT], F32, tag="ndr")
            vh_sb = p_one.tile([128, MT, NC], F32, tag="vh")
            usb = p_one.tile([128, MT, NC], F32, tag="usb")
            zv_sb = p_one.tile([128, MT, NC], BF, tag="zvs")
            zu_sb = p_one.tile([128, MT, NC], BF, tag="zus")
            zvf = p_one.tile([128, KT, NC], BF, tag="zvf")
            zuf = p_one.tile([128, KT, NC], BF, tag="zuf")
            out_sb = p_one.tile([128, MT, NC], F32, tag="osb")

            masks.make_identity(nc, ident)
            nc.sync.dma_start(out=xT_sb, in_=xT[:].rearrange(
                "(kt p) m -> p kt m", p=128))
            nc.sync.dma_start(out=w1_sb, in_=w1[:].rearrange(
                "(kt p) n -> p kt n", p=128))
            nc.sync.dma_start(out=w2_sb, in_=w2h[:].rearrange(
                "(kt p) n -> p kt n", p=128))
            nc.sync.dma_start(out=b2_sb, in_=b2[:].to_broadcast([128, NC]))

            # ---------- adj slab load + column-sum partials ----------
            adj_t = []
            for kt in range(KT):
                a = p_adj.tile([128, RPC], BF, tag="adj")
                nc.sync.dma_start(out=a, in_=adjT[ts(kt, 128), :])
                nc.vector.tensor_reduce(out=csp[:, kt:kt + 1], in_=a,
                                        axis=AX.X, op=OP.add)
                adj_t.append(a)

            # colsum partials -> ReduceScatter -> local d_col slice
            nc.sync.dma_start(
                out=cs_in[:].rearrange("(kt p) -> p kt", p=128), in_=csp)
            nc.gpsimd.collective_compute(
                "ReduceScatter", OP.add, replica_groups=RG,
                ins=[cs_in[:]], outs=[cs_out[:]])
            nc.sync.dma_start(
                out=cloc, in_=cs_out[:].rearrange("(mt p) -> p mt", p=128))
            nc.scalar.activation(out=dcol, in_=cloc, func=AF.Sqrt)
            nc.vector.reciprocal(dcol, dcol)

            # ---------- s = x @ W1 ----------
            with tc.tile_pool(name="ps_s", bufs=2, space="PSUM") as ps_s:
                for mt in range(MT):
                    ps = ps_s.tile([128, NH], F32, tag="ps")
                    for kt in range(FT):
                        nc.tensor.matmul(ps, xT_sb[:, kt, ts(mt, 128)],
                                         w1_sb[:, kt, :],
                                         start=kt == 0, stop=kt == FT - 1)
                    nc.scalar.activation(out=s_sb[:, mt, :], in_=ps,
                                         func=AF.Copy)

            # zs = d_col * s (bf16) with ones column appended
            for mt in range(MT):
                zs_t = p_rot.tile([128, ZW], BF, tag="zs")
                nc.vector.tensor_scalar_mul(zs_t[:, 0:NH], s_sb[:, mt, :],
                                            dcol[:, mt:mt + 1])
                nc.vector.memset(zs_t[:, NH:NH + 1], 1.0)
                nc.vector.memset(zs_t[:, NH + 1:ZW], 0.0)
                nc.sync.dma_start(out=zs_in[ts(mt, 128), :], in_=zs_t)
            nc.gpsimd.collective_compute(
                "AllGather", OP.bypass, replica_groups=RG,
                ins=[zs_in[:]], outs=[zs_out[:]])

            zb_t = []
            for kt in range(KT):
                z = p_zb.tile([128, ZW], BF, tag="zb")
                nc.sync.dma_start(out=z, in_=zs_out[ts(kt, 128), :])
                zb_t.append(z)

            # ---------- passes 1 & 2 (width-258/256), layer-1 epilogue ----
            with ExitStack() as c2:
                pm = c2.enter_context(
                    tc.tile_pool(name="pm", bufs=4, space="PSUM"))
                ptr = c2.enter_context(
                    tc.tile_pool(name="ptr", bufs=2, space="PSUM"))
                pv = c2.enter_context(
                    tc.tile_pool(name="pv", bufs=2, space="PSUM"))

                # pass 1: t' = adj @ zs  (+ rowsum column)
                for g in range(2):
                    mts = range(4 * g, 4 * g + 4)
                    pst = {mt: pm.tile([128, ZW], F32, tag="pm", name=f"pst{mt}")
                           for mt in mts}
                    for kt in range(KT):
                        for mt in mts:
                            nc.tensor.matmul(
                                pst[mt], adj_t[kt][:, ts(mt, 128)], zb_t[kt],
                                start=kt == 0, stop=kt == KT - 1)
                    for mt in mts:
                        # d_row from the ones column; derived scale vectors
                        nc.scalar.activation(out=drow[:, mt:mt + 1],
                                             in_=pst[mt][:, NH:NH + 1],
                                             func=AF.Sqrt)
                        nc.vector.reciprocal(drow[:, mt:mt + 1],
                                             drow[:, mt:mt + 1])
                        nc.vector.tensor_mul(evec[:, mt:mt + 1],
                                             drow[:, mt:mt + 1],
                                             dcol[:, mt:mt + 1])
                        nc.vector.tensor_scalar_mul(n2dr[:, mt:mt + 1],
                                                    drow[:, mt:mt + 1], -2.0)
                        nc.vector.tensor_scalar_mul(ndr[:, mt:mt + 1],
                                                    drow[:, mt:mt + 1], -1.0)
                        # zt = e * t' -> gather operand for pass 2
                        zt_t = p_rot.tile([128, NH], BF, tag="zt")
                        nc.scalar.activation(out=zt_t, in_=pst[mt][:, 0:NH],
                                             func=AF.Copy,
                                             scale=evec[:, mt:mt + 1])
                        nc.sync.dma_start(out=zt_in[ts(mt, 128), :], in_=zt_t)
                        # A = s - d_row*t'  (in place into s_sb)
                        T_t = p_rot.tile([128, NH], F32, tag="T")
                        nc.vector.tensor_scalar_mul(T_t, pst[mt][:, 0:NH],
                                                    drow[:, mt:mt + 1])
                        nc.vector.tensor_sub(s_sb[:, mt, :], s_sb[:, mt, :],
                                             T_t)

                nc.gpsimd.collective_compute(
                    "AllGather", OP.bypass, replica_groups=RG,
                    ins=[zt_in[:]], outs=[zt_out[:]])
                zb2_t = []
                for kt in range(KT):
                    z = p_zb.tile([128, ZW], BF, tag="zb")
                    nc.sync.dma_start(out=z[:, 0:NH],
                                      in_=zt_out[ts(kt, 128), :])
                    zb2_t.append(z)

                # pass 2: r' = adj @ zt ; h' = relu(A - 2*d_row*r') ; v = h@W2
                for g in range(2):
                    mts = range(4 * g, 4 * g + 4)
                    psr = {mt: pm.tile([128, ZW], F32, tag="pm", name=f"psr{mt}")
                           for mt in mts}
                    for kt in range(KT):
                        for mt in mts:
                            nc.tensor.matmul(
                                psr[mt][:, 0:NH], adj_t[kt][:, ts(mt, 128)],
                                zb2_t[kt][:, 0:NH],
                                start=kt == 0, stop=kt == KT - 1)
                    for mt in mts:
                        B_t = p_rot.tile([128, NH], F32, tag="B")
                        nc.scalar.activation(out=B_t, in_=psr[mt][:, 0:NH],
                                             func=AF.Copy,
                                             scale=n2dr[:, mt:mt + 1])
                        nc.vector.tensor_add(B_t, B_t, s_sb[:, mt, :])
                        hp_t = p_rot.tile([128, NH], BF, tag="hp")
                        nc.scalar.activation(out=hp_t, in_=B_t, func=AF.Relu)
                        psv = pv.tile([128, NC], F32, tag="pv")
                        for kh in range(2):
                            pstr = ptr.tile([128, 128], BF, tag="ptr")
                            nc.tensor.transpose(pstr, hp_t[:, ts(kh, 128)],
                                                ident)
                            hT_t = p_rot.tile([128, 128], BF, tag="hT")
                            nc.vector.tensor_copy(hT_t, pstr)
                            nc.tensor.matmul(psv, hT_t, w2_sb[:, kh, :],
                                             start=kh == 0, stop=kh == 1)
                        nc.vector.tensor_scalar_mul(vh_sb[:, mt, :], psv, 0.5)
                        nc.scalar.activation(out=zv_sb[:, mt, :], in_=psv,
                                             func=AF.Copy,
                                             scale=dcol[:, mt:mt + 1])
                nc.sync.dma_start(
                    out=zv_in[:].rearrange("(mt p) c -> p mt c", p=128),
                    in_=zv_sb)

            # ---------- layer-2 narrow passes ----------
            nc.gpsimd.collective_compute(
                "AllGather", OP.bypass, replica_groups=RG,
                ins=[zv_in[:]], outs=[zv_out[:]])
            nc.sync.dma_start(
                out=zvf, in_=zv_out[:].rearrange("(kt p) c -> p kt c", p=128))

            with tc.tile_pool(name="puw", bufs=4, space="PSUM") as puw:
                # u' = adj @ zv
                for g in range(2):
                    mts = range(4 * g, 4 * g + 4)
                    psu = {mt: puw.tile([128, NC], F32, tag="pu",
                                    name=f"psu{mt}") for mt in mts}
                    for kt in range(KT):
                        for mt in mts:
                            nc.tensor.matmul(
                                psu[mt], adj_t[kt][:, ts(mt, 128)],
                                zvf[:, kt, :],
                                start=kt == 0, stop=kt == KT - 1)
                    for mt in mts:
                        nc.vector.tensor_scalar_mul(usb[:, mt, :], psu[mt],
                                                    0.5)
                        nc.scalar.activation(out=zu_sb[:, mt, :], in_=psu[mt],
                                             func=AF.Copy,
                                             scale=evec[:, mt:mt + 1])
                nc.sync.dma_start(
                    out=zu_in[:].rearrange("(mt p) c -> p mt c", p=128),
                    in_=zu_sb)
                nc.gpsimd.collective_compute(
                    "AllGather", OP.bypass, replica_groups=RG,
                    ins=[zu_in[:]], outs=[zu_out[:]])
                nc.sync.dma_start(
                    out=zuf,
                    in_=zu_out[:].rearrange("(kt p) c -> p kt c", p=128))

                # w' = adj @ zu ; out = logsoftmax(0.5v - d_row(0.5u'+w')+b2)
                for g in range(2):
                    mts = range(4 * g, 4 * g + 4)
                    psw = {mt: puw.tile([128, NC], F32, tag="pu",
                                    name=f"psw{mt}") for mt in mts}
                    for kt in range(KT):
                        for mt in mts:
                            nc.tensor.matmul(
                                psw[mt], adj_t[kt][:, ts(mt, 128)],
                                zuf[:, kt, :],
                                start=kt == 0, stop=kt == KT - 1)
                    for mt in mts:
                        G_t = p_rot.tile([128, NC], F32, tag="G")
                        nc.vector.tensor_add(G_t, usb[:, mt, :], psw[mt])
                        nc.vector.tensor_scalar_mul(G_t, G_t,
                                                    ndr[:, mt:mt + 1])
                        nc.vector.tensor_add(G_t, G_t, vh_sb[:, mt, :])
                        nc.vector.tensor_add(G_t, G_t, b2_sb)
                        mx_t = p_rot.tile([128, 1], F32, tag="mx")
                        nc.vector.tensor_reduce(out=mx_t, in_=G_t,
                                                axis=AX.X, op=OP.max)
                        nc.vector.tensor_scalar(G_t, G_t, mx_t, None,
                                                op0=OP.subtract)
                        ex_t = p_rot.tile([128, NC], F32, tag="ex")
                        sm_t = p_rot.tile([128, 1], F32, tag="sm")
                        nc.scalar.activation(out=ex_t, in_=G_t, func=AF.Exp,
                                             accum_out=sm_t)
                        lg_t = p_rot.tile([128, 1], F32, tag="lg")
                        nc.scalar.activation(out=lg_t, in_=sm_t, func=AF.Ln)
                        nc.vector.tensor_scalar(out_sb[:, mt, :], G_t, lg_t,
                                                None, op0=OP.subtract)
                nc.sync.dma_start(
                    out=out[:].rearrange("(mt p) c -> p mt c", p=128),
                    in_=out_sb)

    nc.compile()
    return nc


def _get_nc():
    if "nc" not in _CACHE:
        _CACHE["nc"] = _build()
    return _CACHE["nc"]


def _prep_in_maps(x, adj, W1, W2, b2):
    bf = ml_dtypes.bfloat16
    f32 = np.float32
    x = np.asarray(x, f32)
    adj = np.asarray(adj, f32)
    w1 = np.asarray(W1, f32).astype(bf)
    w2h = (0.5 * np.asarray(W2, f32)).astype(bf)
    b2v = np.asarray(b2, f32).reshape(1, NC)
    in_maps = []
    for i in range(NCORE):
        rows = slice(i * RPC, (i + 1) * RPC)
        in_maps.append({
            "adjT": np.ascontiguousarray(adj[rows, :].T).astype(bf),
            "xT": np.ascontiguousarray(x[rows, :].T).astype(bf),
            "w1": w1, "w2h": w2h, "b2": b2v,
        })
    return in_maps


def _run(x, adj, W1, W2, b2, trace=False):
    from concourse.bass_utils import run_bass_kernel_spmd
    nc = _get_nc()
    in_maps = _prep_in_maps(x, adj, W1, W2, b2)
    res = run_bass_kernel_spmd(nc, in_maps, core_ids=list(range(NCORE)),
                               trace=trace)
    out = np.concatenate([r["out"] for r in res.results], axis=0)
    return out, res


def kernel(x, adj, W1, W2, b2):
    out, _ = _run(x, adj, W1, W2, b2, trace=False)
    return out


# revision 14
# speedup vs baseline: 10682.9554x; 10682.9554x over previous
"""MidGCN forward on 8 Trainium2 NeuronCores (Bass/Tile, SPMD row-sharding).

Math (alpha = 0.5):
  DAD   = d_row * adj * d_col          (d = rsqrt of row/col sums)
  adj_f = (0.5*I - DAD)(I + DAD) = 0.5*I - 0.5*DAD - DAD@DAD
  h     = relu(adj_f @ (x @ W1))
  out   = log_softmax(adj_f @ (h @ W2) + b2)

Key rewrite: never materialize adj_f / DAD@DAD.  With
adjC = adj * d_col (folded into the resident slab once) and
P(y) = adjC @ y, every application is DAD@y = d_row * P(y), so
  adj_f @ y = 0.5*y - d_row*(0.5*P(y) + P(d_row*P(y)))
and each P() is an adjC @ (narrow) matmul.

Sharding: core i holds rows_i = [1024*i, 1024*(i+1)) of adj as the
host-transposed slab adjT_i = adj[rows_i, :].T ([8192, 1024] bf16),
resident in SBUF for all four passes.  Column sums: per-core partials
(free-axis reduces split DVE/ACT, hidden under the DMA load) +
AllReduce; the full d_col then scales the slab in place (global tile
index -> no per-core addressing).  Row sums: ones-vector PE pass over
the raw slab, also hidden under the load (and it warms the PE).  The
x@W1 shard is gathered raw (bf16) while the slab still loads, so pass
1 starts right after the AllReduce.  Narrow activations are scaled
shard-wise (d_row only, purely local) and AllGathered between passes.
Dummy matmul chains bridge the collective gaps to keep the PE HAM
un-throttled.  Output: each core computes log-softmax on its own
[1024, 2] rows; the host concatenates.
"""

import numpy as np
import ml_dtypes

NCORE = 8
N = 8192
NF = 512
NH = 256
NC = 2
RPC = N // NCORE          # rows per core = 1024
KT = N // 128             # 64 contraction tiles
MT = RPC // 128           # 8 output row tiles per core
FT = NF // 128            # 4 k-tiles for x @ W1

_CACHE = {}


def _build(lite=False, sim=False):
    import concourse.bass as bass
    import concourse.mybir as mybir
    import concourse.tile as tile
    from concourse import bacc, masks
    from concourse.bass import ts

    BF = mybir.dt.bfloat16
    F32 = mybir.dt.float32
    AX = mybir.AxisListType
    OP = mybir.AluOpType
    AF = mybir.ActivationFunctionType

    nc = bacc.Bacc("TRN2", target_bir_lowering=False, debug=False,
                   num_devices=NCORE)

    adjT = nc.dram_tensor("adjT", [N, RPC], BF, kind="ExternalInput")
    xT = nc.dram_tensor("xT", [NF, RPC], BF, kind="ExternalInput")
    w1 = nc.dram_tensor("w1", [NF, NH], BF, kind="ExternalInput")
    w2h = nc.dram_tensor("w2h", [NH, NC], BF, kind="ExternalInput")
    b2 = nc.dram_tensor("b2", [1, NC], F32, kind="ExternalInput")
    out = nc.dram_tensor("out", [RPC, NC], F32, kind="ExternalOutput")

    cs_in = nc.dram_tensor("cs_in", [N], F32)
    cs_ar = nc.dram_tensor("cs_ar", [N], F32, addr_space="Shared")
    rs_dram = nc.dram_tensor("rs_dram", [RPC], F32)
    zs_in = nc.dram_tensor("zs_in", [RPC, NH], BF)
    zs_out = nc.dram_tensor("zs_out", [N, NH], BF, addr_space="Shared")
    zt_in = nc.dram_tensor("zt_in", [RPC, NH], BF)
    zt_out = nc.dram_tensor("zt_out", [N, NH], BF, addr_space="Shared")
    zv_in = nc.dram_tensor("zv_in", [RPC, NC], BF)
    zv_out = nc.dram_tensor("zv_out", [N, NC], BF, addr_space="Shared")
    zu_in = nc.dram_tensor("zu_in", [RPC, NC], BF)
    zu_out = nc.dram_tensor("zu_out", [N, NC], BF, addr_space="Shared")
    RG = [list(range(NCORE))]

    if lite:
        # I/O-identical null kernel: measures tunnel/dispatch overhead.
        with tile.TileContext(nc) as tc:
            with tc.tile_pool(name="p0", bufs=1) as p0:
                o = p0.tile([128, MT, NC], F32, tag="o")
                nc.vector.memset(o, 0.0)
                nc.sync.dma_start(
                    out=out[:].rearrange("(mt p) c -> p mt c", p=128), in_=o)
        nc.compile()
        return nc

    with tile.TileContext(nc) as tc:
        from contextlib import ExitStack
        with ExitStack() as ctx:
            p_adj = ctx.enter_context(tc.tile_pool(name="p_adj", bufs=KT))
            p_zb = ctx.enter_context(tc.tile_pool(name="p_zb", bufs=KT))
            p_one = ctx.enter_context(tc.tile_pool(name="p_one", bufs=1))
            p_rot = ctx.enter_context(tc.tile_pool(name="p_rot", bufs=2))

            # ---------- persistent SBUF ----------
            csp = p_one.tile([128, KT], F32, tag="csp")
            s_sb = p_one.tile([128, MT, NH], F32, tag="s")
            xT_sb = p_one.tile([128, FT, RPC], BF, tag="xT")
            w1_sb = p_one.tile([128, FT, NH], BF, tag="w1")
            w2_sb = p_one.tile([128, NC, NC], BF, tag="w2")
            b2_sb = p_one.tile([128, NC], F32, tag="b2")
            ident = p_one.tile([128, 128], BF, tag="ident")
            ones_sb = p_one.tile([128, 1], BF, tag="ones")
            dcolf = p_one.tile([128, KT], F32, tag="dcolf")
            row_sb = p_one.tile([1, RPC], F32, tag="rowsb")
            rloc = p_one.tile([128, MT], F32, tag="rloc")
            drow = p_one.tile([128, MT], F32, tag="drow")
            n2dr = p_one.tile([128, MT], F32, tag="n2dr")
            ndr = p_one.tile([128, MT], F32, tag="ndr")
            vh_sb = p_one.tile([128, MT, NC], F32, tag="vh")
            usb = p_one.tile([128, MT, NC], F32, tag="usb")
            zv_sb = p_one.tile([128, MT, NC], BF, tag="zvs")
            zu_sb = p_one.tile([128, MT, NC], BF, tag="zus")
            zvf = p_one.tile([128, KT, NC], BF, tag="zvf")
            zuf = p_one.tile([128, KT, NC], BF, tag="zuf")
            out_sb = p_one.tile([128, MT, NC], F32, tag="osb")

            masks.make_identity(nc, ident)
            nc.vector.memset(ones_sb, 1.0)
            nc.sync.dma_start(out=xT_sb, in_=xT[:].rearrange(
                "(kt p) m -> p kt m", p=128))
            nc.sync.dma_start(out=w1_sb, in_=w1[:].rearrange(
                "(kt p) n -> p kt n", p=128))
            nc.sync.dma_start(out=w2_sb, in_=w2h[:].rearrange(
                "(kt p) n -> p kt n", p=128))
            nc.sync.dma_start(out=b2_sb, in_=b2[:].to_broadcast([128, NC]))

            # ---------- adj slab load; colsum partials on DVE/ACT ----------
            adj_t = []
            for kt in range(KT):
                a = p_adj.tile([128, RPC], BF, tag="adj", name=f"adj{kt}")
                nc.sync.dma_start(out=a, in_=adjT[ts(kt, 128), :])
                if kt % 2 == 0:
                    nc.vector.tensor_reduce(out=csp[:, kt:kt + 1], in_=a,
                                            axis=AX.X, op=OP.add)
                else:
                    scr = p_rot.tile([128, RPC], BF, tag="scr_a",
                                     name=f"scra{kt}")
                    nc.scalar.activation(out=scr, in_=a, func=AF.Copy,
                                         accum_out=csp[:, kt:kt + 1])
                adj_t.append(a)

            with ExitStack() as c1:
                ps_s = c1.enter_context(
                    tc.tile_pool(name="ps_s", bufs=2, space="PSUM"))
                ps_row = c1.enter_context(
                    tc.tile_pool(name="ps_row", bufs=2, space="PSUM"))
                ps_w0 = c1.enter_context(
                    tc.tile_pool(name="ps_w0", bufs=1, space="PSUM"))

                # ---- s = x @ W1; gather it raw (bf16) while slab loads ----
                for mt in range(MT):
                    ps = ps_s.tile([128, NH], F32, tag="ps")
                    for kt in range(FT):
                        nc.tensor.matmul(ps, xT_sb[:, kt, ts(mt, 128)],
                                         w1_sb[:, kt, :],
                                         start=kt == 0, stop=kt == FT - 1)
                    nc.scalar.activation(out=s_sb[:, mt, :], in_=ps,
                                         func=AF.Copy)
                    zs_t = p_rot.tile([128, NH], BF, tag="zs")
                    nc.vector.tensor_copy(zs_t, ps)
                    nc.sync.dma_start(out=zs_in[ts(mt, 128), :], in_=zs_t)
                if sim:
                    nc.sync.dma_start(out=zs_out[0:RPC, :], in_=zs_in[:])
                else:
                    nc.gpsimd.collective_compute(
                        "AllGather", OP.bypass, replica_groups=RG,
                        ins=[zs_in[:]], outs=[zs_out[:]])
                zb_t = []
                for kt in range(KT):
                    z = p_zb.tile([128, NH], BF, tag="zb", name=f"zb{kt}")
                    nc.sync.dma_start(out=z, in_=zs_out[ts(kt, 128), :])
                    zb_t.append(z)

                # ---- row sums: ones-vector PE pass over the raw slab ----
                prow = [ps_row.tile([1, 512], F32, tag="pr", name=f"pr{j}")
                        for j in range(2)]
                for kt in range(KT):
                    for j in range(2):
                        nc.tensor.matmul(prow[j], ones_sb,
                                         adj_t[kt][:, ts(j, 512)],
                                         start=kt == 0, stop=kt == KT - 1)
                for j in range(2):
                    nc.vector.tensor_copy(row_sb[0:1, ts(j, 512)], prow[j])
                nc.sync.dma_start(out=rs_dram[:], in_=row_sb[0:1, :])
                nc.sync.dma_start(
                    out=rloc,
                    in_=rs_dram[:].rearrange("(mt p) -> p mt", p=128))
                nc.scalar.activation(out=drow, in_=rloc, func=AF.Sqrt)
                nc.vector.reciprocal(drow, drow)
                nc.vector.tensor_scalar_mul(n2dr, drow, -2.0)
                nc.vector.tensor_scalar_mul(ndr, drow, -1.0)

                # keep PE warm while the colsum AllReduce runs
                pw = ps_w0.tile([128, 512], F32, tag="pw")
                for i in range(24):
                    nc.tensor.matmul(pw, ident, adj_t[0][:, 0:512],
                                     start=i == 0, stop=i == 23,
                                     skip_group_check=True)

                # ---- colsum AllReduce -> full d_col -> fold into slab ----
                nc.sync.dma_start(
                    out=cs_in[:].rearrange("(kt p) -> p kt", p=128), in_=csp)
                if sim:
                    nc.sync.dma_start(out=cs_ar[:], in_=cs_in[:])
                else:
                    nc.gpsimd.collective_compute(
                        "AllReduce", OP.add, replica_groups=RG,
                        ins=[cs_in[:]], outs=[cs_ar[:]])
                nc.sync.dma_start(
                    out=dcolf,
                    in_=cs_ar[:].rearrange("(kt p) -> p kt", p=128))
                nc.scalar.activation(out=dcolf, in_=dcolf, func=AF.Sqrt)
                nc.vector.reciprocal(dcolf, dcolf)
                for kt in range(KT):
                    nc.vector.tensor_scalar(adj_t[kt], adj_t[kt],
                                            dcolf[:, kt:kt + 1], None,
                                            op0=OP.mult)

            # ---------- passes 1 & 2 and layer-1 epilogue ----------
            with ExitStack() as c2:
                pm = c2.enter_context(
                    tc.tile_pool(name="pm", bufs=4, space="PSUM"))
                ptr = c2.enter_context(
                    tc.tile_pool(name="ptr", bufs=2, space="PSUM"))
                pv = c2.enter_context(
                    tc.tile_pool(name="pv", bufs=1, space="PSUM"))
                pwm = c2.enter_context(
                    tc.tile_pool(name="pwm", bufs=1, space="PSUM"))

                # pass 1: t' = adjC @ zs
                for g in range(2):
                    mts = range(4 * g, 4 * g + 4)
                    pst = {mt: pm.tile([128, NH], F32, tag="pm",
                                       name=f"pst{mt}") for mt in mts}
                    for kt in range(KT):
                        for mt in mts:
                            nc.tensor.matmul(
                                pst[mt], adj_t[kt][:, ts(mt, 128)], zb_t[kt],
                                start=kt == 0, stop=kt == KT - 1)
                    for mt in mts:
                        # T = d_row * t' ; zt = bf16(T) ; A = s - T (in s_sb)
                        T_t = p_rot.tile([128, NH], F32, tag="T")
                        nc.vector.tensor_scalar(T_t, pst[mt],
                                                drow[:, mt:mt + 1], None,
                                                op0=OP.mult)
                        zt_t = p_rot.tile([128, NH], BF, tag="zt")
                        nc.vector.tensor_copy(zt_t, T_t)
                        nc.sync.dma_start(out=zt_in[ts(mt, 128), :],
                                          in_=zt_t)
                        nc.vector.tensor_sub(s_sb[:, mt, :], s_sb[:, mt, :],
                                             T_t)

                # PE warmth across the zt AllGather
                pw1 = pwm.tile([128, 512], F32, tag="pwm", name="pw1")
                for i in range(44):
                    nc.tensor.matmul(pw1, ident, adj_t[0][:, 0:512],
                                     start=i == 0, stop=i == 43,
                                     skip_group_check=True)

                if sim:
                    nc.sync.dma_start(out=zt_out[0:RPC, :], in_=zt_in[:])
                else:
                    nc.gpsimd.collective_compute(
                        "AllGather", OP.bypass, replica_groups=RG,
                        ins=[zt_in[:]], outs=[zt_out[:]])
                zb2_t = []
                for kt in range(KT):
                    z = p_zb.tile([128, NH], BF, tag="zb", name=f"zb2_{kt}")
                    nc.sync.dma_start(out=z, in_=zt_out[ts(kt, 128), :])
                    zb2_t.append(z)

                # pass 2: r' = adjC @ zt ; h' = relu(A - 2*d_row*r')
                # v = h' @ (W2/2) accumulated via per-tile PE transposes
                for g in range(2):
                    mts = range(4 * g, 4 * g + 4)
                    psr = {mt: pm.tile([128, NH], F32, tag="pm",
                                       name=f"psr{mt}") for mt in mts}
                    for kt in range(KT):
                        for mt in mts:
                            nc.tensor.matmul(
                                psr[mt], adj_t[kt][:, ts(mt, 128)],
                                zb2_t[kt],
                                start=kt == 0, stop=kt == KT - 1)
                    for mt in mts:
                        B_t = p_rot.tile([128, NH], F32, tag="B")
                        nc.vector.tensor_scalar(B_t, psr[mt],
                                                n2dr[:, mt:mt + 1], None,
                                                op0=OP.mult)
                        nc.vector.tensor_add(B_t, B_t, s_sb[:, mt, :])
                        hp_t = p_rot.tile([128, NH], BF, tag="hp")
                        nc.vector.tensor_scalar_max(hp_t, B_t, 0.0)
                        psv = pv.tile([128, NC], F32, tag="pv")
                        for kh in range(2):
                            pstr = ptr.tile([128, 128], BF, tag="ptr")
                            nc.tensor.transpose(pstr, hp_t[:, ts(kh, 128)],
                                                ident)
                            hT_t = p_rot.tile([128, 128], BF, tag="hT")
                            nc.vector.tensor_copy(hT_t, pstr)
                            nc.tensor.matmul(psv, hT_t, w2_sb[:, kh, :],
                                             start=kh == 0, stop=kh == 1)
                        nc.vector.tensor_scalar_mul(vh_sb[:, mt, :], psv, 0.5)
                        nc.vector.tensor_copy(zv_sb[:, mt, :], psv)

                # PE warmth across the zv AllGather
                pw2 = pwm.tile([128, 512], F32, tag="pwm", name="pw2")
                for i in range(40):
                    nc.tensor.matmul(pw2, ident, adj_t[0][:, 0:512],
                                     start=i == 0, stop=i == 39,
                                     skip_group_check=True)

                nc.sync.dma_start(
                    out=zv_in[:].rearrange("(mt p) c -> p mt c", p=128),
                    in_=zv_sb)

            # ---------- layer-2 narrow passes ----------
            if sim:
                nc.sync.dma_start(out=zv_out[0:RPC, :], in_=zv_in[:])
            else:
                nc.gpsimd.collective_compute(
                    "AllGather", OP.bypass, replica_groups=RG,
                    ins=[zv_in[:]], outs=[zv_out[:]])
            nc.sync.dma_start(
                out=zvf, in_=zv_out[:].rearrange("(kt p) c -> p kt c", p=128))

            with ExitStack() as c3:
                puw = c3.enter_context(
                    tc.tile_pool(name="puw", bufs=4, space="PSUM"))
                pwn = c3.enter_context(
                    tc.tile_pool(name="pwn", bufs=1, space="PSUM"))

                # u' = adjC @ zv
                for g in range(2):
                    mts = range(4 * g, 4 * g + 4)
                    psu = {mt: puw.tile([128, NC], F32, tag="pu",
                                        name=f"psu{mt}") for mt in mts}
                    for kt in range(KT):
                        for mt in mts:
                            nc.tensor.matmul(
                                psu[mt], adj_t[kt][:, ts(mt, 128)],
                                zvf[:, kt, :],
                                start=kt == 0, stop=kt == KT - 1)
                    for mt in mts:
                        nc.vector.tensor_scalar_mul(usb[:, mt, :], psu[mt],
                                                    0.5)
                        nc.vector.tensor_scalar(zu_sb[:, mt, :], psu[mt],
                                                drow[:, mt:mt + 1], None,
                                                op0=OP.mult)

                # PE warmth across the zu AllGather
                pw3 = pwn.tile([128, 512], F32, tag="pwn", name="pw3")
                for i in range(40):
                    nc.tensor.matmul(pw3, ident, adj_t[0][:, 0:512],
                                     start=i == 0, stop=i == 39,
                                     skip_group_check=True)

                nc.sync.dma_start(
                    out=zu_in[:].rearrange("(mt p) c -> p mt c", p=128),
                    in_=zu_sb)
                if sim:
                    nc.sync.dma_start(out=zu_out[0:RPC, :], in_=zu_in[:])
                else:
                    nc.gpsimd.collective_compute(
                        "AllGather", OP.bypass, replica_groups=RG,
                        ins=[zu_in[:]], outs=[zu_out[:]])
                nc.sync.dma_start(
                    out=zuf,
                    in_=zu_out[:].rearrange("(kt p) c -> p kt c", p=128))

                # w' = adjC @ zu ; out = logsoftmax(0.5v - drow(0.5u'+w')+b2)
                for g in range(2):
                    mts = range(4 * g, 4 * g + 4)
                    psw = {mt: puw.tile([128, NC], F32, tag="pu",
                                        name=f"psw{mt}") for mt in mts}
                    for kt in range(KT):
                        for mt in mts:
                            nc.tensor.matmul(
                                psw[mt], adj_t[kt][:, ts(mt, 128)],
                                zuf[:, kt, :],
                                start=kt == 0, stop=kt == KT - 1)
                    G_ts, sm_ts = {}, {}
                    for mt in mts:
                        G_t = p_rot.tile([128, NC], F32, tag="G",
                                         name=f"G{mt}", bufs=4)
                        nc.vector.tensor_add(G_t, usb[:, mt, :], psw[mt])
                        nc.vector.tensor_scalar_mul(G_t, G_t,
                                                    ndr[:, mt:mt + 1])
                        nc.vector.tensor_add(G_t, G_t, vh_sb[:, mt, :])
                        nc.vector.tensor_add(G_t, G_t, b2_sb)
                        mx_t = p_rot.tile([128, 1], F32, tag="mx")
                        nc.vector.tensor_reduce(out=mx_t, in_=G_t,
                                                axis=AX.X, op=OP.max)
                        nc.vector.tensor_scalar(G_t, G_t, mx_t, None,
                                                op0=OP.subtract)
                        G_ts[mt] = G_t
                    for mt in mts:
                        ex_t = p_rot.tile([128, NC], F32, tag="ex")
                        sm_t = p_rot.tile([128, 1], F32, tag="sm",
                                          name=f"sm{mt}", bufs=4)
                        nc.scalar.activation(out=ex_t, in_=G_ts[mt],
                                             func=AF.Exp, accum_out=sm_t)
                        sm_ts[mt] = sm_t
                    for mt in mts:
                        lg_t = p_rot.tile([128, 1], F32, tag="lg")
                        nc.scalar.activation(out=lg_t, in_=sm_ts[mt],
                                             func=AF.Ln)
                        nc.vector.tensor_scalar(out_sb[:, mt, :], G_ts[mt],
                                                lg_t, None, op0=OP.subtract)
                nc.sync.dma_start(
                    out=out[:].rearrange("(mt p) c -> p mt c", p=128),
                    in_=out_sb)

    nc.compile()
    return nc


def _get_nc(lite=False):
    key = "nc_lite" if lite else "nc"
    if key not in _CACHE:
        _CACHE[key] = _build(lite=lite)
    return _CACHE[key]


def _prep_in_maps(x, adj, W1, W2, b2):
    bf = ml_dtypes.bfloat16
    f32 = np.float32
    x = np.asarray(x, f32)
    adj = np.asarray(adj, f32)
    w1 = np.asarray(W1, f32).astype(bf)
    w2h = (0.5 * np.asarray(W2, f32)).astype(bf)
    b2v = np.asarray(b2, f32).reshape(1, NC)
    in_maps = []
    for i in range(NCORE):
        rows = slice(i * RPC, (i + 1) * RPC)
        in_maps.append({
            "adjT": adj[rows, :].T.astype(bf),   # one fused copy+cast
            "xT": x[rows, :].T.astype(bf),
            "w1": w1, "w2h": w2h, "b2": b2v,
        })
    return in_maps


def _run(x, adj, W1, W2, b2, trace=False, lite=False, in_maps=None):
    from concourse.bass_utils import run_bass_kernel_spmd
    nc = _get_nc(lite=lite)
    if in_maps is None:
        in_maps = _prep_in_maps(x, adj, W1, W2, b2)
    res = run_bass_kernel_spmd(nc, in_maps, core_ids=list(range(NCORE)),
                               trace=trace)
    out = np.concatenate([r["out"] for r in res.results], axis=0)
    return out, res


def kernel(x, adj, W1, W2, b2):
    out, _ = _run(x, adj, W1, W2, b2, trace=False)
    return out


# revision 15
# speedup vs baseline: 11382.0568x; 1.0654x over previous
"""MidGCN forward on 8 Trainium2 NeuronCores (Bass/Tile, SPMD row-sharding).

Math (alpha = 0.5):
  DAD   = d_row * adj * d_col          (d = rsqrt of row/col sums)
  adj_f = (0.5*I - DAD)(I + DAD) = 0.5*I - 0.5*DAD - DAD@DAD
  h     = relu(adj_f @ (x @ W1))
  out   = log_softmax(adj_f @ (h @ W2) + b2)

Key rewrite: never materialize adj_f / DAD@DAD.  With
adjC = adj * d_col (folded into the resident slab once) and
P(y) = adjC @ y, every application is DAD@y = d_row * P(y), so
  adj_f @ y = 0.5*y - d_row*(0.5*P(y) + P(d_row*P(y)))
and each P() is an adjC @ (narrow) matmul.

Sharding: core i holds rows_i = [1024*i, 1024*(i+1)) of adj as the
host-transposed slab adjT_i = adj[rows_i, :].T ([8192, 1024] bf16),
resident in SBUF for all four passes.  Column sums: per-core partials
(free-axis reduces split DVE/ACT, hidden under the DMA load) +
AllReduce; the full d_col then scales the slab in place (global tile
index -> no per-core addressing).  Row sums: ones-vector PE pass over
the raw slab, also hidden under the load (and it warms the PE).  The
x@W1 shard is gathered raw (bf16) while the slab still loads, so pass
1 starts right after the AllReduce.  Narrow activations are scaled
shard-wise (d_row only, purely local) and AllGathered between passes.
Dummy matmul chains bridge the collective gaps to keep the PE HAM
un-throttled.  Output: each core computes log-softmax on its own
[1024, 2] rows; the host concatenates.
"""

import numpy as np
import ml_dtypes

NCORE = 8
N = 8192
NF = 512
NH = 256
NC = 2
RPC = N // NCORE          # rows per core = 1024
KT = N // 128             # 64 contraction tiles
MT = RPC // 128           # 8 output row tiles per core
FT = NF // 128            # 4 k-tiles for x @ W1

_CACHE = {}


def _build(lite=False, sim=False):
    import concourse.bass as bass
    import concourse.mybir as mybir
    import concourse.tile as tile
    from concourse import bacc, masks
    from concourse.bass import ts

    BF = mybir.dt.bfloat16
    F32 = mybir.dt.float32
    AX = mybir.AxisListType
    OP = mybir.AluOpType
    AF = mybir.ActivationFunctionType

    nc = bacc.Bacc("TRN2", target_bir_lowering=False, debug=False,
                   num_devices=NCORE)

    adjT = nc.dram_tensor("adjT", [N, RPC], BF, kind="ExternalInput")
    xT = nc.dram_tensor("xT", [NF, RPC], BF, kind="ExternalInput")
    w1 = nc.dram_tensor("w1", [NF, NH], BF, kind="ExternalInput")
    w2h = nc.dram_tensor("w2h", [NH, NC], BF, kind="ExternalInput")
    b2 = nc.dram_tensor("b2", [1, NC], F32, kind="ExternalInput")
    out = nc.dram_tensor("out", [RPC, NC], F32, kind="ExternalOutput")

    cs_in = nc.dram_tensor("cs_in", [N], F32)
    cs_ar = nc.dram_tensor("cs_ar", [N], F32, addr_space="Shared")
    rs_dram = nc.dram_tensor("rs_dram", [RPC], F32)
    zs_in = nc.dram_tensor("zs_in", [RPC, NH], BF)
    zs_out = nc.dram_tensor("zs_out", [N, NH], BF, addr_space="Shared")
    zt_in = nc.dram_tensor("zt_in", [RPC, NH], BF)
    zt_out = nc.dram_tensor("zt_out", [N, NH], BF, addr_space="Shared")
    zv_in = nc.dram_tensor("zv_in", [RPC, NC], BF)
    zv_out = nc.dram_tensor("zv_out", [N, NC], BF, addr_space="Shared")
    zu_in = nc.dram_tensor("zu_in", [RPC, NC], BF)
    zu_out = nc.dram_tensor("zu_out", [N, NC], BF, addr_space="Shared")
    RG = [list(range(NCORE))]

    if lite:
        # I/O-identical null kernel: measures tunnel/dispatch overhead.
        with tile.TileContext(nc) as tc:
            with tc.tile_pool(name="p0", bufs=1) as p0:
                o = p0.tile([128, MT, NC], F32, tag="o")
                nc.vector.memset(o, 0.0)
                nc.sync.dma_start(
                    out=out[:].rearrange("(mt p) c -> p mt c", p=128), in_=o)
        nc.compile()
        return nc

    with tile.TileContext(nc) as tc:
        from contextlib import ExitStack
        with ExitStack() as ctx:
            p_adj = ctx.enter_context(tc.tile_pool(name="p_adj", bufs=KT))
            p_zb = ctx.enter_context(tc.tile_pool(name="p_zb", bufs=KT))
            p_one = ctx.enter_context(tc.tile_pool(name="p_one", bufs=1))
            p_rot = ctx.enter_context(tc.tile_pool(name="p_rot", bufs=2))

            # ---------- persistent SBUF ----------
            csp = p_one.tile([128, KT], F32, tag="csp")
            s_sb = p_one.tile([128, MT, NH], F32, tag="s")
            xT_sb = p_one.tile([128, FT, RPC], BF, tag="xT")
            w1_sb = p_one.tile([128, FT, NH], BF, tag="w1")
            w2_sb = p_one.tile([128, NC, NC], BF, tag="w2")
            b2_sb = p_one.tile([128, NC], F32, tag="b2")
            ident = p_one.tile([128, 128], BF, tag="ident")
            ones_sb = p_one.tile([128, 1], BF, tag="ones")
            dcolf = p_one.tile([128, KT], F32, tag="dcolf")
            row_sb = p_one.tile([1, RPC], F32, tag="rowsb")
            rloc = p_one.tile([128, MT], F32, tag="rloc")
            drow = p_one.tile([128, MT], F32, tag="drow")
            n2dr = p_one.tile([128, MT], F32, tag="n2dr")
            ndr = p_one.tile([128, MT], F32, tag="ndr")
            vh_sb = p_one.tile([128, MT, NC], F32, tag="vh")
            usb = p_one.tile([128, MT, NC], F32, tag="usb")
            zv_sb = p_one.tile([128, MT, NC], BF, tag="zvs")
            zu_sb = p_one.tile([128, MT, NC], BF, tag="zus")
            zvf = p_one.tile([128, KT, NC], BF, tag="zvf")
            zuf = p_one.tile([128, KT, NC], BF, tag="zuf")
            out_sb = p_one.tile([128, MT, NC], F32, tag="osb")

            masks.make_identity(nc, ident)
            nc.vector.memset(ones_sb, 1.0)
            nc.sync.dma_start(out=xT_sb, in_=xT[:].rearrange(
                "(kt p) m -> p kt m", p=128))
            nc.sync.dma_start(out=w1_sb, in_=w1[:].rearrange(
                "(kt p) n -> p kt n", p=128))
            nc.sync.dma_start(out=w2_sb, in_=w2h[:].rearrange(
                "(kt p) n -> p kt n", p=128))
            nc.sync.dma_start(out=b2_sb, in_=b2[:].to_broadcast([128, NC]))

            # ---------- adj slab load; colsum partials on DVE/ACT ----------
            adj_t = []
            for kt in range(KT):
                a = p_adj.tile([128, RPC], BF, tag="adj", name=f"adj{kt}")
                nc.sync.dma_start(out=a, in_=adjT[ts(kt, 128), :])
                if kt % 2 == 0:
                    nc.vector.tensor_reduce(out=csp[:, kt:kt + 1], in_=a,
                                            axis=AX.X, op=OP.add)
                else:
                    scr = p_rot.tile([128, RPC], BF, tag="scr_a",
                                     name=f"scra{kt}")
                    nc.scalar.activation(out=scr, in_=a, func=AF.Copy,
                                         accum_out=csp[:, kt:kt + 1])
                adj_t.append(a)

            with ExitStack() as c1:
                ps_s = c1.enter_context(
                    tc.tile_pool(name="ps_s", bufs=2, space="PSUM"))
                ps_row = c1.enter_context(
                    tc.tile_pool(name="ps_row", bufs=2, space="PSUM"))
                ps_w0 = c1.enter_context(
                    tc.tile_pool(name="ps_w0", bufs=1, space="PSUM"))

                # ---- s = x @ W1; gather it raw (bf16) while slab loads ----
                for mt in range(MT):
                    ps = ps_s.tile([128, NH], F32, tag="ps")
                    for kt in range(FT):
                        nc.tensor.matmul(ps, xT_sb[:, kt, ts(mt, 128)],
                                         w1_sb[:, kt, :],
                                         start=kt == 0, stop=kt == FT - 1)
                    nc.scalar.activation(out=s_sb[:, mt, :], in_=ps,
                                         func=AF.Copy)
                    zs_t = p_rot.tile([128, NH], BF, tag="zs", bufs=4)
                    nc.vector.tensor_copy(zs_t, ps)
                    nc.sync.dma_start(out=zs_in[ts(mt, 128), :], in_=zs_t)
                if sim:
                    nc.sync.dma_start(out=zs_out[0:RPC, :], in_=zs_in[:])
                else:
                    nc.gpsimd.collective_compute(
                        "AllGather", OP.bypass, replica_groups=RG,
                        ins=[zs_in[:]], outs=[zs_out[:]])
                zb_t = []
                for kt in range(KT):
                    z = p_zb.tile([128, NH], BF, tag="zb", name=f"zb{kt}")
                    nc.sync.dma_start(out=z, in_=zs_out[ts(kt, 128), :])
                    zb_t.append(z)

                # ---- row sums: ones-vector PE pass over the raw slab ----
                prow = [ps_row.tile([1, 512], F32, tag="pr", name=f"pr{j}")
                        for j in range(2)]
                for kt in range(KT):
                    for j in range(2):
                        nc.tensor.matmul(prow[j], ones_sb,
                                         adj_t[kt][:, ts(j, 512)],
                                         start=kt == 0, stop=kt == KT - 1)
                for j in range(2):
                    nc.vector.tensor_copy(row_sb[0:1, ts(j, 512)], prow[j])
                nc.sync.dma_start(out=rs_dram[:], in_=row_sb[0:1, :])
                nc.sync.dma_start(
                    out=rloc,
                    in_=rs_dram[:].rearrange("(mt p) -> p mt", p=128))
                nc.scalar.activation(out=drow, in_=rloc, func=AF.Sqrt)
                nc.vector.reciprocal(drow, drow)
                nc.vector.tensor_scalar_mul(n2dr, drow, -2.0)
                nc.vector.tensor_scalar_mul(ndr, drow, -1.0)

                # keep PE warm while the colsum AllReduce runs
                pw = ps_w0.tile([128, 512], F32, tag="pw")
                for i in range(24):
                    nc.tensor.matmul(pw, ident, adj_t[0][:, 0:512],
                                     start=i == 0, stop=i == 23,
                                     skip_group_check=True)

                # ---- colsum AllReduce -> full d_col -> fold into slab ----
                nc.sync.dma_start(
                    out=cs_in[:].rearrange("(kt p) -> p kt", p=128), in_=csp)
                if sim:
                    nc.sync.dma_start(out=cs_ar[:], in_=cs_in[:])
                else:
                    nc.gpsimd.collective_compute(
                        "AllReduce", OP.add, replica_groups=RG,
                        ins=[cs_in[:]], outs=[cs_ar[:]])
                nc.sync.dma_start(
                    out=dcolf,
                    in_=cs_ar[:].rearrange("(kt p) -> p kt", p=128))
                nc.scalar.activation(out=dcolf, in_=dcolf, func=AF.Sqrt)
                nc.vector.reciprocal(dcolf, dcolf)
                for kt in range(KT):
                    nc.vector.tensor_scalar(adj_t[kt], adj_t[kt],
                                            dcolf[:, kt:kt + 1], None,
                                            op0=OP.mult)

            # ---------- passes 1 & 2 and layer-1 epilogue ----------
            with ExitStack() as c2:
                pm = c2.enter_context(
                    tc.tile_pool(name="pm", bufs=4, space="PSUM"))
                ptr = c2.enter_context(
                    tc.tile_pool(name="ptr", bufs=2, space="PSUM"))
                pv = c2.enter_context(
                    tc.tile_pool(name="pv", bufs=1, space="PSUM"))
                pwm = c2.enter_context(
                    tc.tile_pool(name="pwm", bufs=1, space="PSUM"))

                # pass 1: t' = adjC @ zs
                for g in range(2):
                    mts = range(4 * g, 4 * g + 4)
                    pst = {mt: pm.tile([128, NH], F32, tag="pm",
                                       name=f"pst{mt}") for mt in mts}
                    for kt in range(KT):
                        for mt in mts:
                            nc.tensor.matmul(
                                pst[mt], adj_t[kt][:, ts(mt, 128)], zb_t[kt],
                                start=kt == 0, stop=kt == KT - 1)
                    for mt in mts:
                        # T = d_row * t' ; zt = bf16(T) ; A = s - T (in s_sb)
                        T_t = p_rot.tile([128, NH], F32, tag="T", bufs=4)
                        nc.vector.tensor_scalar(T_t, pst[mt],
                                                drow[:, mt:mt + 1], None,
                                                op0=OP.mult)
                        zt_t = p_rot.tile([128, NH], BF, tag="zt", bufs=4)
                        nc.vector.tensor_copy(zt_t, T_t)
                        nc.sync.dma_start(out=zt_in[ts(mt, 128), :],
                                          in_=zt_t)
                        nc.vector.tensor_sub(s_sb[:, mt, :], s_sb[:, mt, :],
                                             T_t)

                # PE warmth across the zt AllGather
                pw1 = pwm.tile([128, 512], F32, tag="pwm", name="pw1")
                for i in range(44):
                    nc.tensor.matmul(pw1, ident, adj_t[0][:, 0:512],
                                     start=i == 0, stop=i == 43,
                                     skip_group_check=True)

                if sim:
                    nc.sync.dma_start(out=zt_out[0:RPC, :], in_=zt_in[:])
                else:
                    nc.gpsimd.collective_compute(
                        "AllGather", OP.bypass, replica_groups=RG,
                        ins=[zt_in[:]], outs=[zt_out[:]])
                zb2_t = []
                for kt in range(KT):
                    z = p_zb.tile([128, NH], BF, tag="zb", name=f"zb2_{kt}")
                    nc.sync.dma_start(out=z, in_=zt_out[ts(kt, 128), :])
                    zb2_t.append(z)

                # pass 2: r' = adjC @ zt ; h' = relu(A - 2*d_row*r')
                # v = h' @ (W2/2) accumulated via per-tile PE transposes
                for g in range(2):
                    mts = range(4 * g, 4 * g + 4)
                    psr = {mt: pm.tile([128, NH], F32, tag="pm",
                                       name=f"psr{mt}") for mt in mts}
                    for kt in range(KT):
                        for mt in mts:
                            nc.tensor.matmul(
                                psr[mt], adj_t[kt][:, ts(mt, 128)],
                                zb2_t[kt],
                                start=kt == 0, stop=kt == KT - 1)
                    for mt in mts:
                        B_t = p_rot.tile([128, NH], F32, tag="B", bufs=4)
                        nc.vector.tensor_scalar(B_t, psr[mt],
                                                n2dr[:, mt:mt + 1], None,
                                                op0=OP.mult)
                        nc.vector.tensor_add(B_t, B_t, s_sb[:, mt, :])
                        hp_t = p_rot.tile([128, NH], BF, tag="hp", bufs=4)
                        nc.vector.tensor_scalar_max(hp_t, B_t, 0.0)
                        psv = pv.tile([128, NC], F32, tag="pv")
                        for kh in range(2):
                            pstr = ptr.tile([128, 128], BF, tag="ptr")
                            nc.tensor.transpose(pstr, hp_t[:, ts(kh, 128)],
                                                ident)
                            hT_t = p_rot.tile([128, 128], BF, tag="hT",
                                              bufs=3)
                            nc.scalar.activation(out=hT_t, in_=pstr,
                                                 func=AF.Copy)
                            nc.tensor.matmul(psv, hT_t, w2_sb[:, kh, :],
                                             start=kh == 0, stop=kh == 1)
                        nc.scalar.activation(out=vh_sb[:, mt, :], in_=psv,
                                             func=AF.Copy, scale=0.5)
                        nc.vector.tensor_copy(zv_sb[:, mt, :], psv)

                # PE warmth across the zv AllGather
                pw2 = pwm.tile([128, 512], F32, tag="pwm", name="pw2")
                for i in range(40):
                    nc.tensor.matmul(pw2, ident, adj_t[0][:, 0:512],
                                     start=i == 0, stop=i == 39,
                                     skip_group_check=True)

                nc.sync.dma_start(
                    out=zv_in[:].rearrange("(mt p) c -> p mt c", p=128),
                    in_=zv_sb)

            # ---------- layer-2 narrow passes ----------
            if sim:
                nc.sync.dma_start(out=zv_out[0:RPC, :], in_=zv_in[:])
            else:
                nc.gpsimd.collective_compute(
                    "AllGather", OP.bypass, replica_groups=RG,
                    ins=[zv_in[:]], outs=[zv_out[:]])
            nc.sync.dma_start(
                out=zvf, in_=zv_out[:].rearrange("(kt p) c -> p kt c", p=128))

            with ExitStack() as c3:
                puw = c3.enter_context(
                    tc.tile_pool(name="puw", bufs=4, space="PSUM"))
                pwn = c3.enter_context(
                    tc.tile_pool(name="pwn", bufs=1, space="PSUM"))

                # u' = adjC @ zv
                for g in range(2):
                    mts = range(4 * g, 4 * g + 4)
                    psu = {mt: puw.tile([128, NC], F32, tag="pu",
                                        name=f"psu{mt}") for mt in mts}
                    for kt in range(KT):
                        for mt in mts:
                            nc.tensor.matmul(
                                psu[mt], adj_t[kt][:, ts(mt, 128)],
                                zvf[:, kt, :],
                                start=kt == 0, stop=kt == KT - 1)
                    for mt in mts:
                        nc.vector.tensor_scalar_mul(usb[:, mt, :], psu[mt],
                                                    0.5)
                        nc.vector.tensor_scalar(zu_sb[:, mt, :], psu[mt],
                                                drow[:, mt:mt + 1], None,
                                                op0=OP.mult)

                # PE warmth across the zu AllGather
                pw3 = pwn.tile([128, 512], F32, tag="pwn", name="pw3")
                for i in range(40):
                    nc.tensor.matmul(pw3, ident, adj_t[0][:, 0:512],
                                     start=i == 0, stop=i == 39,
                                     skip_group_check=True)

                nc.sync.dma_start(
                    out=zu_in[:].rearrange("(mt p) c -> p mt c", p=128),
                    in_=zu_sb)
                if sim:
                    nc.sync.dma_start(out=zu_out[0:RPC, :], in_=zu_in[:])
                else:
                    nc.gpsimd.collective_compute(
                        "AllGather", OP.bypass, replica_groups=RG,
                        ins=[zu_in[:]], outs=[zu_out[:]])
                nc.sync.dma_start(
                    out=zuf,
                    in_=zu_out[:].rearrange("(kt p) c -> p kt c", p=128))

                # w' = adjC @ zu ; out = logsoftmax(0.5v - drow(0.5u'+w')+b2)
                for g in range(2):
                    mts = range(4 * g, 4 * g + 4)
                    psw = {mt: puw.tile([128, NC], F32, tag="pu",
                                        name=f"psw{mt}") for mt in mts}
                    for kt in range(KT):
                        for mt in mts:
                            nc.tensor.matmul(
                                psw[mt], adj_t[kt][:, ts(mt, 128)],
                                zuf[:, kt, :],
                                start=kt == 0, stop=kt == KT - 1)
                    G_ts, sm_ts = {}, {}
                    for mt in mts:
                        G_t = p_rot.tile([128, NC], F32, tag="G",
                                         name=f"G{mt}", bufs=4)
                        nc.vector.tensor_add(G_t, usb[:, mt, :], psw[mt])
                        nc.vector.tensor_scalar_mul(G_t, G_t,
                                                    ndr[:, mt:mt + 1])
                        nc.vector.tensor_add(G_t, G_t, vh_sb[:, mt, :])
                        nc.vector.tensor_add(G_t, G_t, b2_sb)
                        mx_t = p_rot.tile([128, 1], F32, tag="mx")
                        nc.vector.tensor_reduce(out=mx_t, in_=G_t,
                                                axis=AX.X, op=OP.max)
                        nc.vector.tensor_scalar(G_t, G_t, mx_t, None,
                                                op0=OP.subtract)
                        G_ts[mt] = G_t
                    for mt in mts:
                        ex_t = p_rot.tile([128, NC], F32, tag="ex")
                        sm_t = p_rot.tile([128, 1], F32, tag="sm",
                                          name=f"sm{mt}", bufs=4)
                        nc.scalar.activation(out=ex_t, in_=G_ts[mt],
                                             func=AF.Exp, accum_out=sm_t)
                        sm_ts[mt] = sm_t
                    for mt in mts:
                        lg_t = p_rot.tile([128, 1], F32, tag="lg")
                        nc.scalar.activation(out=lg_t, in_=sm_ts[mt],
                                             func=AF.Ln)
                        nc.vector.tensor_scalar(out_sb[:, mt, :], G_ts[mt],
                                                lg_t, None, op0=OP.subtract)
                nc.sync.dma_start(
                    out=out[:].rearrange("(mt p) c -> p mt c", p=128),
                    in_=out_sb)

    nc.compile()
    return nc


def _get_nc(lite=False):
    key = "nc_lite" if lite else "nc"
    if key not in _CACHE:
        _CACHE[key] = _build(lite=lite)
    return _CACHE[key]


def _prep_in_maps(x, adj, W1, W2, b2):
    bf = ml_dtypes.bfloat16
    f32 = np.float32
    x = np.asarray(x, f32)
    adj = np.asarray(adj, f32)
    w1 = np.asarray(W1, f32).astype(bf)
    w2h = (0.5 * np.asarray(W2, f32)).astype(bf)
    b2v = np.asarray(b2, f32).reshape(1, NC)
    in_maps = []
    for i in range(NCORE):
        rows = slice(i * RPC, (i + 1) * RPC)
        in_maps.append({
            "adjT": adj[rows, :].T.astype(bf),   # one fused copy+cast
            "xT": x[rows, :].T.astype(bf),
            "w1": w1, "w2h": w2h, "b2": b2v,
        })
    return in_maps


def _run(x, adj, W1, W2, b2, trace=False, lite=False, in_maps=None):
    from concourse.bass_utils import run_bass_kernel_spmd
    nc = _get_nc(lite=lite)
    if in_maps is None:
        in_maps = _prep_in_maps(x, adj, W1, W2, b2)
    res = run_bass_kernel_spmd(nc, in_maps, core_ids=list(range(NCORE)),
                               trace=trace)
    out = np.concatenate([r["out"] for r in res.results], axis=0)
    return out, res


def kernel(x, adj, W1, W2, b2):
    out, _ = _run(x, adj, W1, W2, b2, trace=False)
    return out


# revision 16
# speedup vs baseline: 11403.1129x; 1.0018x over previous
"""MidGCN forward on 8 Trainium2 NeuronCores (Bass/Tile, SPMD row-sharding).

Math (alpha = 0.5):
  DAD   = d_row * adj * d_col          (d = rsqrt of row/col sums)
  adj_f = (0.5*I - DAD)(I + DAD) = 0.5*I - 0.5*DAD - DAD@DAD
  h     = relu(adj_f @ (x @ W1))
  out   = log_softmax(adj_f @ (h @ W2) + b2)

Key rewrite: never materialize adj_f / DAD@DAD.  With
adjC = adj * d_col (folded into the resident slab once) and
P(y) = adjC @ y, every application is DAD@y = d_row * P(y), so
  adj_f @ y = 0.5*y - d_row*(0.5*P(y) + P(d_row*P(y)))
and each P() is an adjC @ (narrow) matmul.

Sharding: core i holds rows_i = [1024*i, 1024*(i+1)) of adj as the
host-transposed slab adjT_i = adj[rows_i, :].T ([8192, 1024] bf16),
resident in SBUF for all four passes.  Column sums: per-core partials
(free-axis reduces split DVE/ACT, hidden under the DMA load) +
AllReduce; the full d_col then scales the slab in place (global tile
index -> no per-core addressing).  Row sums: ones-vector PE pass over
the raw slab, also hidden under the load (and it warms the PE).  The
x@W1 shard is gathered raw (bf16) while the slab still loads, so pass
1 starts right after the AllReduce.  Narrow activations are scaled
shard-wise (d_row only, purely local) and AllGathered between passes.
Dummy matmul chains bridge the collective gaps to keep the PE HAM
un-throttled.  Output: each core computes log-softmax on its own
[1024, 2] rows; the host concatenates.
"""

import numpy as np
import ml_dtypes

NCORE = 8
N = 8192
NF = 512
NH = 256
NC = 2
RPC = N // NCORE          # rows per core = 1024
KT = N // 128             # 64 contraction tiles
MT = RPC // 128           # 8 output row tiles per core
FT = NF // 128            # 4 k-tiles for x @ W1

_CACHE = {}


def _build(lite=False, sim=False):
    import concourse.bass as bass
    import concourse.mybir as mybir
    import concourse.tile as tile
    from concourse import bacc, masks
    from concourse.bass import ts

    BF = mybir.dt.bfloat16
    F32 = mybir.dt.float32
    AX = mybir.AxisListType
    OP = mybir.AluOpType
    AF = mybir.ActivationFunctionType

    nc = bacc.Bacc("TRN2", target_bir_lowering=False, debug=False,
                   num_devices=NCORE)

    adjT = nc.dram_tensor("adjT", [N, RPC], BF, kind="ExternalInput")
    xT = nc.dram_tensor("xT", [NF, RPC], BF, kind="ExternalInput")
    w1 = nc.dram_tensor("w1", [NF, NH], BF, kind="ExternalInput")
    w2h = nc.dram_tensor("w2h", [NH, NC], BF, kind="ExternalInput")
    b2 = nc.dram_tensor("b2", [1, NC], F32, kind="ExternalInput")
    out = nc.dram_tensor("out", [RPC, NC], F32, kind="ExternalOutput")

    cs_in = nc.dram_tensor("cs_in", [N], F32)
    cs_ar = nc.dram_tensor("cs_ar", [N], F32, addr_space="Shared")
    rs_dram = nc.dram_tensor("rs_dram", [RPC], F32)
    zs_in = nc.dram_tensor("zs_in", [RPC, NH], BF)
    zs_out = nc.dram_tensor("zs_out", [N, NH], BF, addr_space="Shared")
    zt_in = nc.dram_tensor("zt_in", [RPC, NH], BF)
    zt_out = nc.dram_tensor("zt_out", [N, NH], BF, addr_space="Shared")
    zv_in = nc.dram_tensor("zv_in", [RPC, NC], BF)
    zv_out = nc.dram_tensor("zv_out", [N, NC], BF, addr_space="Shared")
    zu_in = nc.dram_tensor("zu_in", [RPC, NC], BF)
    zu_out = nc.dram_tensor("zu_out", [N, NC], BF, addr_space="Shared")
    RG = [list(range(NCORE))]

    if lite:
        # I/O-identical null kernel: measures tunnel/dispatch overhead.
        with tile.TileContext(nc) as tc:
            with tc.tile_pool(name="p0", bufs=1) as p0:
                o = p0.tile([128, MT, NC], F32, tag="o")
                nc.vector.memset(o, 0.0)
                nc.sync.dma_start(
                    out=out[:].rearrange("(mt p) c -> p mt c", p=128), in_=o)
        nc.compile()
        return nc

    with tile.TileContext(nc) as tc:
        from contextlib import ExitStack
        with ExitStack() as ctx:
            p_adj = ctx.enter_context(tc.tile_pool(name="p_adj", bufs=KT))
            p_zb = ctx.enter_context(tc.tile_pool(name="p_zb", bufs=KT))
            p_one = ctx.enter_context(tc.tile_pool(name="p_one", bufs=1))
            p_rot = ctx.enter_context(tc.tile_pool(name="p_rot", bufs=2))

            # ---------- persistent SBUF ----------
            csp = p_one.tile([128, KT], F32, tag="csp")
            s_sb = p_one.tile([128, MT, NH], F32, tag="s")
            xT_sb = p_one.tile([128, FT, RPC], BF, tag="xT")
            w1_sb = p_one.tile([128, FT, NH], BF, tag="w1")
            w2_sb = p_one.tile([128, NC, NC], BF, tag="w2")
            b2_sb = p_one.tile([128, NC], F32, tag="b2")
            ident = p_one.tile([128, 128], BF, tag="ident")
            ones_sb = p_one.tile([128, 1], BF, tag="ones")
            dcolf = p_one.tile([128, KT], F32, tag="dcolf")
            row_sb = p_one.tile([1, RPC], F32, tag="rowsb")
            rloc = p_one.tile([128, MT], F32, tag="rloc")
            drow = p_one.tile([128, MT], F32, tag="drow")
            n2dr = p_one.tile([128, MT], F32, tag="n2dr")
            ndr = p_one.tile([128, MT], F32, tag="ndr")
            vh_sb = p_one.tile([128, MT, NC], F32, tag="vh")
            usb = p_one.tile([128, MT, NC], F32, tag="usb")
            zv_sb = p_one.tile([128, MT, NC], BF, tag="zvs")
            zu_sb = p_one.tile([128, MT, NC], BF, tag="zus")
            zvf = p_one.tile([128, KT, NC], BF, tag="zvf")
            zuf = p_one.tile([128, KT, NC], BF, tag="zuf")
            out_sb = p_one.tile([128, MT, NC], F32, tag="osb")

            masks.make_identity(nc, ident)
            nc.vector.memset(ones_sb, 1.0)
            nc.sync.dma_start(out=xT_sb, in_=xT[:].rearrange(
                "(kt p) m -> p kt m", p=128))
            nc.sync.dma_start(out=w1_sb, in_=w1[:].rearrange(
                "(kt p) n -> p kt n", p=128))
            nc.sync.dma_start(out=w2_sb, in_=w2h[:].rearrange(
                "(kt p) n -> p kt n", p=128))
            nc.sync.dma_start(out=b2_sb, in_=b2[:].to_broadcast([128, NC]))

            # ---------- adj slab load; colsum partials on DVE/ACT ----------
            adj_t = []
            for kt in range(KT):
                a = p_adj.tile([128, RPC], BF, tag="adj", name=f"adj{kt}")
                nc.sync.dma_start(out=a, in_=adjT[ts(kt, 128), :])
                if kt % 2 == 0:
                    nc.vector.tensor_reduce(out=csp[:, kt:kt + 1], in_=a,
                                            axis=AX.X, op=OP.add)
                else:
                    scr = p_rot.tile([128, RPC], BF, tag="scr_a",
                                     name=f"scra{kt}")
                    nc.scalar.activation(out=scr, in_=a, func=AF.Copy,
                                         accum_out=csp[:, kt:kt + 1])
                adj_t.append(a)

            with ExitStack() as c1:
                ps_s = c1.enter_context(
                    tc.tile_pool(name="ps_s", bufs=2, space="PSUM"))
                ps_row = c1.enter_context(
                    tc.tile_pool(name="ps_row", bufs=2, space="PSUM"))
                ps_w0 = c1.enter_context(
                    tc.tile_pool(name="ps_w0", bufs=1, space="PSUM"))

                # ---- s = x @ W1; gather it raw (bf16) while slab loads ----
                for mt in range(MT):
                    ps = ps_s.tile([128, NH], F32, tag="ps")
                    for kt in range(FT):
                        nc.tensor.matmul(ps, xT_sb[:, kt, ts(mt, 128)],
                                         w1_sb[:, kt, :],
                                         start=kt == 0, stop=kt == FT - 1)
                    nc.scalar.activation(out=s_sb[:, mt, :], in_=ps,
                                         func=AF.Copy)
                    zs_t = p_rot.tile([128, NH], BF, tag="zs", bufs=4)
                    nc.vector.tensor_copy(zs_t, ps)
                    nc.sync.dma_start(out=zs_in[ts(mt, 128), :], in_=zs_t)
                if sim:
                    nc.sync.dma_start(out=zs_out[0:RPC, :], in_=zs_in[:])
                else:
                    nc.gpsimd.collective_compute(
                        "AllGather", OP.bypass, replica_groups=RG,
                        ins=[zs_in[:]], outs=[zs_out[:]])
                zb_t = []
                for kt in range(KT):
                    z = p_zb.tile([128, NH], BF, tag="zb", name=f"zb{kt}")
                    nc.sync.dma_start(out=z, in_=zs_out[ts(kt, 128), :])
                    zb_t.append(z)

                # ---- row sums: ones-vector PE pass over the raw slab ----
                prow = [ps_row.tile([1, 512], F32, tag="pr", name=f"pr{j}")
                        for j in range(2)]
                for kt in range(KT):
                    for j in range(2):
                        nc.tensor.matmul(prow[j], ones_sb,
                                         adj_t[kt][:, ts(j, 512)],
                                         start=kt == 0, stop=kt == KT - 1)
                for j in range(2):
                    nc.vector.tensor_copy(row_sb[0:1, ts(j, 512)], prow[j])
                nc.sync.dma_start(out=rs_dram[:], in_=row_sb[0:1, :])
                nc.sync.dma_start(
                    out=rloc,
                    in_=rs_dram[:].rearrange("(mt p) -> p mt", p=128))
                nc.scalar.activation(out=drow, in_=rloc, func=AF.Sqrt)
                nc.vector.reciprocal(drow, drow)
                nc.vector.tensor_scalar_mul(n2dr, drow, -2.0)
                nc.vector.tensor_scalar_mul(ndr, drow, -1.0)

                # keep PE warm while the colsum AllReduce runs
                pw = ps_w0.tile([128, 512], F32, tag="pw")
                for i in range(24):
                    nc.tensor.matmul(pw, ident, adj_t[KT - 1][:, 0:512],
                                     start=i == 0, stop=i == 23,
                                     skip_group_check=True)

                # ---- colsum AllReduce -> full d_col -> fold into slab ----
                nc.sync.dma_start(
                    out=cs_in[:].rearrange("(kt p) -> p kt", p=128), in_=csp)
                if sim:
                    nc.sync.dma_start(out=cs_ar[:], in_=cs_in[:])
                else:
                    nc.gpsimd.collective_compute(
                        "AllReduce", OP.add, replica_groups=RG,
                        ins=[cs_in[:]], outs=[cs_ar[:]])
                nc.sync.dma_start(
                    out=dcolf,
                    in_=cs_ar[:].rearrange("(kt p) -> p kt", p=128))
                nc.scalar.activation(out=dcolf, in_=dcolf, func=AF.Sqrt)
                nc.vector.reciprocal(dcolf, dcolf)
                for kt in range(KT):
                    nc.vector.tensor_scalar(adj_t[kt], adj_t[kt],
                                            dcolf[:, kt:kt + 1], None,
                                            op0=OP.mult)

            # ---------- passes 1 & 2 and layer-1 epilogue ----------
            with ExitStack() as c2:
                pm = c2.enter_context(
                    tc.tile_pool(name="pm", bufs=4, space="PSUM"))
                ptr = c2.enter_context(
                    tc.tile_pool(name="ptr", bufs=2, space="PSUM"))
                pv = c2.enter_context(
                    tc.tile_pool(name="pv", bufs=1, space="PSUM"))
                pwm = c2.enter_context(
                    tc.tile_pool(name="pwm", bufs=1, space="PSUM"))

                # pass 1: t' = adjC @ zs
                for g in range(2):
                    mts = range(4 * g, 4 * g + 4)
                    pst = {mt: pm.tile([128, NH], F32, tag="pm",
                                       name=f"pst{mt}") for mt in mts}
                    for kt in range(KT):
                        for mt in mts:
                            nc.tensor.matmul(
                                pst[mt], adj_t[kt][:, ts(mt, 128)], zb_t[kt],
                                start=kt == 0, stop=kt == KT - 1)
                    for mt in mts:
                        # T = d_row * t' ; zt = bf16(T) ; A = s - T (in s_sb)
                        T_t = p_rot.tile([128, NH], F32, tag="T", bufs=4)
                        nc.vector.tensor_scalar(T_t, pst[mt],
                                                drow[:, mt:mt + 1], None,
                                                op0=OP.mult)
                        zt_t = p_rot.tile([128, NH], BF, tag="zt", bufs=4)
                        nc.vector.tensor_copy(zt_t, T_t)
                        nc.sync.dma_start(out=zt_in[ts(mt, 128), :],
                                          in_=zt_t)
                        nc.vector.tensor_sub(s_sb[:, mt, :], s_sb[:, mt, :],
                                             T_t)

                # PE warmth across the zt AllGather
                pw1 = pwm.tile([128, 512], F32, tag="pwm", name="pw1")
                for i in range(44):
                    nc.tensor.matmul(pw1, ident, adj_t[0][:, 0:512],
                                     start=i == 0, stop=i == 43,
                                     skip_group_check=True)

                if sim:
                    nc.sync.dma_start(out=zt_out[0:RPC, :], in_=zt_in[:])
                else:
                    nc.gpsimd.collective_compute(
                        "AllGather", OP.bypass, replica_groups=RG,
                        ins=[zt_in[:]], outs=[zt_out[:]])
                zb2_t = []
                for kt in range(KT):
                    z = p_zb.tile([128, NH], BF, tag="zb", name=f"zb2_{kt}")
                    nc.sync.dma_start(out=z, in_=zt_out[ts(kt, 128), :])
                    zb2_t.append(z)

                # pass 2: r' = adjC @ zt ; h' = relu(A - 2*d_row*r')
                # v = h' @ (W2/2) accumulated via per-tile PE transposes
                for g in range(2):
                    mts = range(4 * g, 4 * g + 4)
                    psr = {mt: pm.tile([128, NH], F32, tag="pm",
                                       name=f"psr{mt}") for mt in mts}
                    for kt in range(KT):
                        for mt in mts:
                            nc.tensor.matmul(
                                psr[mt], adj_t[kt][:, ts(mt, 128)],
                                zb2_t[kt],
                                start=kt == 0, stop=kt == KT - 1)
                    for mt in mts:
                        B_t = p_rot.tile([128, NH], F32, tag="B", bufs=4)
                        nc.vector.tensor_scalar(B_t, psr[mt],
                                                n2dr[:, mt:mt + 1], None,
                                                op0=OP.mult)
                        nc.vector.tensor_add(B_t, B_t, s_sb[:, mt, :])
                        hp_t = p_rot.tile([128, NH], BF, tag="hp", bufs=4)
                        nc.vector.tensor_scalar_max(hp_t, B_t, 0.0)
                        psv = pv.tile([128, NC], F32, tag="pv")
                        for kh in range(2):
                            pstr = ptr.tile([128, 128], BF, tag="ptr")
                            nc.tensor.transpose(pstr, hp_t[:, ts(kh, 128)],
                                                ident)
                            hT_t = p_rot.tile([128, 128], BF, tag="hT",
                                              bufs=3)
                            nc.scalar.activation(out=hT_t, in_=pstr,
                                                 func=AF.Copy)
                            nc.tensor.matmul(psv, hT_t, w2_sb[:, kh, :],
                                             start=kh == 0, stop=kh == 1)
                        nc.scalar.activation(out=vh_sb[:, mt, :], in_=psv,
                                             func=AF.Copy, scale=0.5)
                        nc.vector.tensor_copy(zv_sb[:, mt, :], psv)

                # PE warmth across the zv AllGather
                pw2 = pwm.tile([128, 512], F32, tag="pwm", name="pw2")
                for i in range(40):
                    nc.tensor.matmul(pw2, ident, adj_t[0][:, 0:512],
                                     start=i == 0, stop=i == 39,
                                     skip_group_check=True)

                nc.sync.dma_start(
                    out=zv_in[:].rearrange("(mt p) c -> p mt c", p=128),
                    in_=zv_sb)

            # ---------- layer-2 narrow passes ----------
            if sim:
                nc.sync.dma_start(out=zv_out[0:RPC, :], in_=zv_in[:])
            else:
                nc.gpsimd.collective_compute(
                    "AllGather", OP.bypass, replica_groups=RG,
                    ins=[zv_in[:]], outs=[zv_out[:]])
            nc.sync.dma_start(
                out=zvf, in_=zv_out[:].rearrange("(kt p) c -> p kt c", p=128))

            with ExitStack() as c3:
                puw = c3.enter_context(
                    tc.tile_pool(name="puw", bufs=6, space="PSUM"))
                pwn = c3.enter_context(
                    tc.tile_pool(name="pwn", bufs=1, space="PSUM"))

                # u' = adjC @ zv
                for g in range(2):
                    mts = range(4 * g, 4 * g + 4)
                    psu = {mt: puw.tile([128, NC], F32, tag="pu",
                                        name=f"psu{mt}") for mt in mts}
                    for kt in range(KT):
                        for mt in mts:
                            nc.tensor.matmul(
                                psu[mt], adj_t[kt][:, ts(mt, 128)],
                                zvf[:, kt, :],
                                start=kt == 0, stop=kt == KT - 1)
                    for mt in mts:
                        nc.vector.tensor_scalar_mul(usb[:, mt, :], psu[mt],
                                                    0.5)
                        nc.vector.tensor_scalar(zu_sb[:, mt, :], psu[mt],
                                                drow[:, mt:mt + 1], None,
                                                op0=OP.mult)

                # PE warmth across the zu AllGather
                pw3 = pwn.tile([128, 512], F32, tag="pwn", name="pw3")
                for i in range(40):
                    nc.tensor.matmul(pw3, ident, adj_t[0][:, 0:512],
                                     start=i == 0, stop=i == 39,
                                     skip_group_check=True)

                nc.sync.dma_start(
                    out=zu_in[:].rearrange("(mt p) c -> p mt c", p=128),
                    in_=zu_sb)
                if sim:
                    nc.sync.dma_start(out=zu_out[0:RPC, :], in_=zu_in[:])
                else:
                    nc.gpsimd.collective_compute(
                        "AllGather", OP.bypass, replica_groups=RG,
                        ins=[zu_in[:]], outs=[zu_out[:]])
                nc.sync.dma_start(
                    out=zuf,
                    in_=zu_out[:].rearrange("(kt p) c -> p kt c", p=128))

                # w' = adjC @ zu ; out = logsoftmax(0.5v - drow(0.5u'+w')+b2)
                for g in range(2):
                    mts = range(4 * g, 4 * g + 4)
                    psw = {mt: puw.tile([128, NC], F32, tag="pu",
                                        name=f"psw{mt}") for mt in mts}
                    for kt in range(KT):
                        for mt in mts:
                            nc.tensor.matmul(
                                psw[mt], adj_t[kt][:, ts(mt, 128)],
                                zuf[:, kt, :],
                                start=kt == 0, stop=kt == KT - 1)
                    G_ts, sm_ts = {}, {}
                    for mt in mts:
                        G_t = p_rot.tile([128, NC], F32, tag="G",
                                         name=f"G{mt}", bufs=4)
                        nc.vector.tensor_add(G_t, usb[:, mt, :], psw[mt])
                        nc.vector.tensor_scalar_mul(G_t, G_t,
                                                    ndr[:, mt:mt + 1])
                        nc.vector.tensor_add(G_t, G_t, vh_sb[:, mt, :])
                        nc.vector.tensor_add(G_t, G_t, b2_sb)
                        mx_t = p_rot.tile([128, 1], F32, tag="mx")
                        nc.vector.tensor_reduce(out=mx_t, in_=G_t,
                                                axis=AX.X, op=OP.max)
                        nc.vector.tensor_scalar(G_t, G_t, mx_t, None,
                                                op0=OP.subtract)
                        G_ts[mt] = G_t
                    for mt in mts:
                        ex_t = p_rot.tile([128, NC], F32, tag="ex")
                        sm_t = p_rot.tile([128, 1], F32, tag="sm",
                                          name=f"sm{mt}", bufs=4)
                        nc.scalar.activation(out=ex_t, in_=G_ts[mt],
                                             func=AF.Exp, accum_out=sm_t)
                        sm_ts[mt] = sm_t
                    for mt in mts:
                        lg_t = p_rot.tile([128, 1], F32, tag="lg")
                        nc.scalar.activation(out=lg_t, in_=sm_ts[mt],
                                             func=AF.Ln)
                        nc.vector.tensor_scalar(out_sb[:, mt, :], G_ts[mt],
                                                lg_t, None, op0=OP.subtract)
                nc.sync.dma_start(
                    out=out[:].rearrange("(mt p) c -> p mt c", p=128),
                    in_=out_sb)

    nc.compile()
    return nc


def _get_nc(lite=False):
    key = "nc_lite" if lite else "nc"
    if key not in _CACHE:
        _CACHE[key] = _build(lite=lite)
    return _CACHE[key]


def _prep_in_maps(x, adj, W1, W2, b2):
    bf = ml_dtypes.bfloat16
    f32 = np.float32
    x = np.asarray(x, f32)
    adj = np.asarray(adj, f32)
    w1 = np.asarray(W1, f32).astype(bf)
    w2h = (0.5 * np.asarray(W2, f32)).astype(bf)
    b2v = np.asarray(b2, f32).reshape(1, NC)
    in_maps = []
    for i in range(NCORE):
        rows = slice(i * RPC, (i + 1) * RPC)
        in_maps.append({
            "adjT": adj[rows, :].T.astype(bf),   # one fused copy+cast
            "xT": x[rows, :].T.astype(bf),
            "w1": w1, "w2h": w2h, "b2": b2v,
        })
    return in_maps


def _run(x, adj, W1, W2, b2, trace=False, lite=False, in_maps=None):
    from concourse.bass_utils import run_bass_kernel_spmd
    nc = _get_nc(lite=lite)
    if in_maps is None:
        in_maps = _prep_in_maps(x, adj, W1, W2, b2)
    res = run_bass_kernel_spmd(nc, in_maps, core_ids=list(range(NCORE)),
                               trace=trace)
    out = np.concatenate([r["out"] for r in res.results], axis=0)
    return out, res


def kernel(x, adj, W1, W2, b2):
    out, _ = _run(x, adj, W1, W2, b2, trace=False)
    return out


# revision 17
# speedup vs baseline: 11434.1648x; 1.0027x over previous
"""MidGCN forward on 8 Trainium2 NeuronCores (Bass/Tile, SPMD row-sharding).

Math (alpha = 0.5):
  DAD   = d_row * adj * d_col          (d = rsqrt of row/col sums)
  adj_f = (0.5*I - DAD)(I + DAD) = 0.5*I - 0.5*DAD - DAD@DAD
  h     = relu(adj_f @ (x @ W1))
  out   = log_softmax(adj_f @ (h @ W2) + b2)

Key rewrite: never materialize adj_f / DAD@DAD.  With
adjC = adj * d_col (folded into the resident slab once) and
P(y) = adjC @ y, every application is DAD@y = d_row * P(y), so
  adj_f @ y = 0.5*y - d_row*(0.5*P(y) + P(d_row*P(y)))
and each P() is an adjC @ (narrow) matmul.

Sharding: core i holds rows_i = [1024*i, 1024*(i+1)) of adj as the
host-transposed slab adjT_i = adj[rows_i, :].T ([8192, 1024] bf16),
resident in SBUF for all four passes.  Column sums: per-core partials
(free-axis reduces split DVE/ACT, hidden under the DMA load) +
AllReduce; the full d_col then scales the slab in place (global tile
index -> no per-core addressing).  Row sums: ones-vector PE pass over
the raw slab, also hidden under the load (and it warms the PE).  The
x@W1 shard is gathered raw (bf16) while the slab still loads, so pass
1 starts right after the AllReduce.  Narrow activations are scaled
shard-wise (d_row only, purely local) and AllGathered between passes.
Dummy matmul chains bridge the collective gaps to keep the PE HAM
un-throttled.  Output: each core computes log-softmax on its own
[1024, 2] rows; the host concatenates.
"""

import numpy as np
import ml_dtypes

NCORE = 8
N = 8192
NF = 512
NH = 256
NC = 2
RPC = N // NCORE          # rows per core = 1024
KT = N // 128             # 64 contraction tiles
MT = RPC // 128           # 8 output row tiles per core
FT = NF // 128            # 4 k-tiles for x @ W1

_CACHE = {}


def _build(lite=False, sim=False):
    import concourse.bass as bass
    import concourse.mybir as mybir
    import concourse.tile as tile
    from concourse import bacc, masks
    from concourse.bass import ts

    BF = mybir.dt.bfloat16
    F32 = mybir.dt.float32
    AX = mybir.AxisListType
    OP = mybir.AluOpType
    AF = mybir.ActivationFunctionType

    nc = bacc.Bacc("TRN2", target_bir_lowering=False, debug=False,
                   num_devices=NCORE)

    adjT = nc.dram_tensor("adjT", [N, RPC], BF, kind="ExternalInput")
    xT = nc.dram_tensor("xT", [NF, RPC], BF, kind="ExternalInput")
    w1 = nc.dram_tensor("w1", [NF, NH], BF, kind="ExternalInput")
    w2h = nc.dram_tensor("w2h", [NH, NC], BF, kind="ExternalInput")
    b2 = nc.dram_tensor("b2", [1, NC], F32, kind="ExternalInput")
    out = nc.dram_tensor("out", [RPC, NC], F32, kind="ExternalOutput")

    cs_in = nc.dram_tensor("cs_in", [N], F32)
    cs_ar = nc.dram_tensor("cs_ar", [N], F32, addr_space="Shared")
    rs_dram = nc.dram_tensor("rs_dram", [RPC], F32)
    zs_in = nc.dram_tensor("zs_in", [RPC, NH], BF)
    zs_out = nc.dram_tensor("zs_out", [N, NH], BF, addr_space="Shared")
    zt_in = nc.dram_tensor("zt_in", [RPC, NH], BF)
    zt_out = nc.dram_tensor("zt_out", [N, NH], BF, addr_space="Shared")
    zv_in = nc.dram_tensor("zv_in", [RPC, NC], BF)
    zv_out = nc.dram_tensor("zv_out", [N, NC], BF, addr_space="Shared")
    zu_in = nc.dram_tensor("zu_in", [RPC, NC], BF)
    zu_out = nc.dram_tensor("zu_out", [N, NC], BF, addr_space="Shared")
    RG = [list(range(NCORE))]

    if lite:
        # I/O-identical null kernel: measures tunnel/dispatch overhead.
        with tile.TileContext(nc) as tc:
            with tc.tile_pool(name="p0", bufs=1) as p0:
                o = p0.tile([128, MT, NC], F32, tag="o")
                nc.vector.memset(o, 0.0)
                nc.sync.dma_start(
                    out=out[:].rearrange("(mt p) c -> p mt c", p=128), in_=o)
        nc.compile()
        return nc

    with tile.TileContext(nc) as tc:
        from contextlib import ExitStack
        with ExitStack() as ctx:
            p_adj = ctx.enter_context(tc.tile_pool(name="p_adj", bufs=KT))
            p_zb = ctx.enter_context(tc.tile_pool(name="p_zb", bufs=KT))
            p_one = ctx.enter_context(tc.tile_pool(name="p_one", bufs=1))
            p_rot = ctx.enter_context(tc.tile_pool(name="p_rot", bufs=2))

            # ---------- persistent SBUF ----------
            csp = p_one.tile([128, KT], F32, tag="csp")
            s_sb = p_one.tile([128, MT, NH], F32, tag="s")
            xT_sb = p_one.tile([128, FT, RPC], BF, tag="xT")
            w1_sb = p_one.tile([128, FT, NH], BF, tag="w1")
            w2_sb = p_one.tile([128, NC, NC], BF, tag="w2")
            b2_sb = p_one.tile([128, NC], F32, tag="b2")
            ident = p_one.tile([128, 128], BF, tag="ident")
            ones_sb = p_one.tile([128, 1], BF, tag="ones")
            dcolf = p_one.tile([128, KT], F32, tag="dcolf")
            row_sb = p_one.tile([1, RPC], F32, tag="rowsb")
            rloc = p_one.tile([128, MT], F32, tag="rloc")
            drow = p_one.tile([128, MT], F32, tag="drow")
            n2dr = p_one.tile([128, MT], F32, tag="n2dr")
            ndr = p_one.tile([128, MT], F32, tag="ndr")
            vh_sb = p_one.tile([128, MT, NC], F32, tag="vh")
            usb = p_one.tile([128, MT, NC], F32, tag="usb")
            zv_sb = p_one.tile([128, MT, NC], BF, tag="zvs")
            zu_sb = p_one.tile([128, MT, NC], BF, tag="zus")
            zvf = p_one.tile([128, KT, NC], BF, tag="zvf")
            zuf = p_one.tile([128, KT, NC], BF, tag="zuf")
            out_sb = p_one.tile([128, MT, NC], F32, tag="osb")

            masks.make_identity(nc, ident)
            nc.vector.memset(ones_sb, 1.0)
            nc.sync.dma_start(out=xT_sb, in_=xT[:].rearrange(
                "(kt p) m -> p kt m", p=128))
            nc.sync.dma_start(out=w1_sb, in_=w1[:].rearrange(
                "(kt p) n -> p kt n", p=128))
            nc.sync.dma_start(out=w2_sb, in_=w2h[:].rearrange(
                "(kt p) n -> p kt n", p=128))
            nc.sync.dma_start(out=b2_sb, in_=b2[:].to_broadcast([128, NC]))

            # ---------- adj slab load; colsum partials on DVE/ACT ----------
            adj_t = []
            for kt in range(KT):
                a = p_adj.tile([128, RPC], BF, tag="adj", name=f"adj{kt}")
                nc.sync.dma_start(out=a, in_=adjT[ts(kt, 128), :])
                if kt % 2 == 0:
                    nc.vector.tensor_reduce(out=csp[:, kt:kt + 1], in_=a,
                                            axis=AX.X, op=OP.add)
                else:
                    scr = p_rot.tile([128, RPC], BF, tag="scr_a",
                                     name=f"scra{kt}")
                    nc.scalar.activation(out=scr, in_=a, func=AF.Copy,
                                         accum_out=csp[:, kt:kt + 1])
                adj_t.append(a)

            with ExitStack() as c1:
                ps_s = c1.enter_context(
                    tc.tile_pool(name="ps_s", bufs=2, space="PSUM"))
                ps_row = c1.enter_context(
                    tc.tile_pool(name="ps_row", bufs=2, space="PSUM"))
                ps_w0 = c1.enter_context(
                    tc.tile_pool(name="ps_w0", bufs=1, space="PSUM"))

                # ---- s = x @ W1; gather it raw (bf16) while slab loads ----
                for mt in range(MT):
                    ps = ps_s.tile([128, NH], F32, tag="ps")
                    for kt in range(FT):
                        nc.tensor.matmul(ps, xT_sb[:, kt, ts(mt, 128)],
                                         w1_sb[:, kt, :],
                                         start=kt == 0, stop=kt == FT - 1)
                    nc.scalar.activation(out=s_sb[:, mt, :], in_=ps,
                                         func=AF.Copy)
                    zs_t = p_rot.tile([128, NH], BF, tag="zs", bufs=4)
                    nc.vector.tensor_copy(zs_t, ps)
                    nc.sync.dma_start(out=zs_in[ts(mt, 128), :], in_=zs_t)
                if sim:
                    nc.sync.dma_start(out=zs_out[0:RPC, :], in_=zs_in[:])
                else:
                    nc.gpsimd.collective_compute(
                        "AllGather", OP.bypass, replica_groups=RG,
                        ins=[zs_in[:]], outs=[zs_out[:]])
                zb_t = []
                for kt in range(KT):
                    z = p_zb.tile([128, NH], BF, tag="zb", name=f"zb{kt}")
                    nc.sync.dma_start(out=z, in_=zs_out[ts(kt, 128), :])
                    zb_t.append(z)

                # ---- row sums: ones-vector PE pass over the raw slab ----
                prow = [ps_row.tile([1, 512], F32, tag="pr", name=f"pr{j}")
                        for j in range(2)]
                for kt in range(KT):
                    for j in range(2):
                        nc.tensor.matmul(prow[j], ones_sb,
                                         adj_t[kt][:, ts(j, 512)],
                                         start=kt == 0, stop=kt == KT - 1)
                for j in range(2):
                    nc.vector.tensor_copy(row_sb[0:1, ts(j, 512)], prow[j])
                nc.sync.dma_start(out=rs_dram[:], in_=row_sb[0:1, :])
                nc.sync.dma_start(
                    out=rloc,
                    in_=rs_dram[:].rearrange("(mt p) -> p mt", p=128))
                nc.scalar.activation(out=drow, in_=rloc, func=AF.Sqrt)
                nc.vector.reciprocal(drow, drow)
                nc.vector.tensor_scalar_mul(n2dr, drow, -2.0)
                nc.vector.tensor_scalar_mul(ndr, drow, -1.0)

                # keep PE warm while the colsum AllReduce runs
                pw = ps_w0.tile([128, 512], F32, tag="pw")
                for i in range(24):
                    nc.tensor.matmul(pw, ident, adj_t[KT - 1][:, 0:512],
                                     start=i == 0, stop=i == 23,
                                     skip_group_check=True)

                # ---- colsum AllReduce -> full d_col -> fold into slab ----
                nc.sync.dma_start(
                    out=cs_in[:].rearrange("(kt p) -> p kt", p=128), in_=csp)
                if sim:
                    nc.sync.dma_start(out=cs_ar[:], in_=cs_in[:])
                else:
                    nc.gpsimd.collective_compute(
                        "AllReduce", OP.add, replica_groups=RG,
                        ins=[cs_in[:]], outs=[cs_ar[:]])
                nc.sync.dma_start(
                    out=dcolf,
                    in_=cs_ar[:].rearrange("(kt p) -> p kt", p=128))
                nc.scalar.activation(out=dcolf, in_=dcolf, func=AF.Sqrt)
                nc.vector.reciprocal(dcolf, dcolf)
                for kt in range(KT):
                    nc.vector.tensor_scalar(adj_t[kt], adj_t[kt],
                                            dcolf[:, kt:kt + 1], None,
                                            op0=OP.mult)

            # ---------- passes 1 & 2 and layer-1 epilogue ----------
            with ExitStack() as c2:
                pm = c2.enter_context(
                    tc.tile_pool(name="pm", bufs=4, space="PSUM"))
                ptr = c2.enter_context(
                    tc.tile_pool(name="ptr", bufs=2, space="PSUM"))
                pv = c2.enter_context(
                    tc.tile_pool(name="pv", bufs=1, space="PSUM"))
                pwm = c2.enter_context(
                    tc.tile_pool(name="pwm", bufs=1, space="PSUM"))

                # pass 1: t' = adjC @ zs
                for g in range(2):
                    mts = range(4 * g, 4 * g + 4)
                    pst = {mt: pm.tile([128, NH], F32, tag="pm",
                                       name=f"pst{mt}") for mt in mts}
                    for kt in range(KT):
                        for mt in mts:
                            nc.tensor.matmul(
                                pst[mt], adj_t[kt][:, ts(mt, 128)], zb_t[kt],
                                start=kt == 0, stop=kt == KT - 1)
                    for mt in mts:
                        # T = d_row * t' ; zt = bf16(T) ; A = s - T (in s_sb)
                        T_t = p_rot.tile([128, NH], F32, tag="T", bufs=4)
                        nc.vector.tensor_scalar(T_t, pst[mt],
                                                drow[:, mt:mt + 1], None,
                                                op0=OP.mult)
                        zt_t = p_rot.tile([128, NH], BF, tag="zt", bufs=4)
                        nc.vector.tensor_copy(zt_t, T_t)
                        nc.sync.dma_start(out=zt_in[ts(mt, 128), :],
                                          in_=zt_t)
                        nc.vector.tensor_sub(s_sb[:, mt, :], s_sb[:, mt, :],
                                             T_t)
                        zt_last = zt_t

                # PE warmth across the zt AllGather, anchored to the last
                # zt tile so the chain spans the collective window
                pw1 = pwm.tile([128, 512], F32, tag="pwm", name="pw1")
                nc.tensor.matmul(pw1[:, 0:NH], ident, zt_last,
                                 start=True, stop=False,
                                 skip_group_check=True)
                for i in range(43):
                    nc.tensor.matmul(pw1, ident, adj_t[0][:, 0:512],
                                     start=False, stop=i == 42,
                                     skip_group_check=True)

                if sim:
                    nc.sync.dma_start(out=zt_out[0:RPC, :], in_=zt_in[:])
                else:
                    nc.gpsimd.collective_compute(
                        "AllGather", OP.bypass, replica_groups=RG,
                        ins=[zt_in[:]], outs=[zt_out[:]])
                zb2_t = []
                for kt in range(KT):
                    z = p_zb.tile([128, NH], BF, tag="zb", name=f"zb2_{kt}")
                    nc.sync.dma_start(out=z, in_=zt_out[ts(kt, 128), :])
                    zb2_t.append(z)

                # pass 2: r' = adjC @ zt ; h' = relu(A - 2*d_row*r')
                # v = h' @ (W2/2) accumulated via per-tile PE transposes
                for g in range(2):
                    mts = range(4 * g, 4 * g + 4)
                    psr = {mt: pm.tile([128, NH], F32, tag="pm",
                                       name=f"psr{mt}") for mt in mts}
                    for kt in range(KT):
                        for mt in mts:
                            nc.tensor.matmul(
                                psr[mt], adj_t[kt][:, ts(mt, 128)],
                                zb2_t[kt],
                                start=kt == 0, stop=kt == KT - 1)
                    for mt in mts:
                        B_t = p_rot.tile([128, NH], F32, tag="B", bufs=4)
                        nc.vector.tensor_scalar(B_t, psr[mt],
                                                n2dr[:, mt:mt + 1], None,
                                                op0=OP.mult)
                        nc.vector.tensor_add(B_t, B_t, s_sb[:, mt, :])
                        hp_t = p_rot.tile([128, NH], BF, tag="hp", bufs=4)
                        nc.vector.tensor_scalar_max(hp_t, B_t, 0.0)
                        psv = pv.tile([128, NC], F32, tag="pv")
                        for kh in range(2):
                            pstr = ptr.tile([128, 128], BF, tag="ptr")
                            nc.tensor.transpose(pstr, hp_t[:, ts(kh, 128)],
                                                ident)
                            hT_t = p_rot.tile([128, 128], BF, tag="hT",
                                              bufs=3)
                            nc.scalar.activation(out=hT_t, in_=pstr,
                                                 func=AF.Copy)
                            nc.tensor.matmul(psv, hT_t, w2_sb[:, kh, :],
                                             start=kh == 0, stop=kh == 1)
                        nc.scalar.activation(out=vh_sb[:, mt, :], in_=psv,
                                             func=AF.Copy, scale=0.5)
                        nc.vector.tensor_copy(zv_sb[:, mt, :], psv)

                # PE warmth across the zv AllGather, anchored to zv_sb
                pw2 = pwm.tile([128, 512], F32, tag="pwm", name="pw2")
                nc.tensor.matmul(pw2[:, 0:MT * NC], ident,
                                 zv_sb[:].rearrange("p a b -> p (a b)"),
                                 start=True, stop=False,
                                 skip_group_check=True)
                for i in range(39):
                    nc.tensor.matmul(pw2, ident, adj_t[0][:, 0:512],
                                     start=False, stop=i == 38,
                                     skip_group_check=True)

                nc.sync.dma_start(
                    out=zv_in[:].rearrange("(mt p) c -> p mt c", p=128),
                    in_=zv_sb)

            # ---------- layer-2 narrow passes ----------
            if sim:
                nc.sync.dma_start(out=zv_out[0:RPC, :], in_=zv_in[:])
            else:
                nc.gpsimd.collective_compute(
                    "AllGather", OP.bypass, replica_groups=RG,
                    ins=[zv_in[:]], outs=[zv_out[:]])
            nc.sync.dma_start(
                out=zvf, in_=zv_out[:].rearrange("(kt p) c -> p kt c", p=128))

            with ExitStack() as c3:
                puw = c3.enter_context(
                    tc.tile_pool(name="puw", bufs=6, space="PSUM"))
                pwn = c3.enter_context(
                    tc.tile_pool(name="pwn", bufs=1, space="PSUM"))

                # u' = adjC @ zv
                for g in range(2):
                    mts = range(4 * g, 4 * g + 4)
                    psu = {mt: puw.tile([128, NC], F32, tag="pu",
                                        name=f"psu{mt}") for mt in mts}
                    for kt in range(KT):
                        for mt in mts:
                            nc.tensor.matmul(
                                psu[mt], adj_t[kt][:, ts(mt, 128)],
                                zvf[:, kt, :],
                                start=kt == 0, stop=kt == KT - 1)
                    for mt in mts:
                        nc.vector.tensor_scalar_mul(usb[:, mt, :], psu[mt],
                                                    0.5)
                        nc.vector.tensor_scalar(zu_sb[:, mt, :], psu[mt],
                                                drow[:, mt:mt + 1], None,
                                                op0=OP.mult)

                # PE warmth across the zu AllGather, anchored to zu_sb
                pw3 = pwn.tile([128, 512], F32, tag="pwn", name="pw3")
                nc.tensor.matmul(pw3[:, 0:MT * NC], ident,
                                 zu_sb[:].rearrange("p a b -> p (a b)"),
                                 start=True, stop=False,
                                 skip_group_check=True)
                for i in range(39):
                    nc.tensor.matmul(pw3, ident, adj_t[0][:, 0:512],
                                     start=False, stop=i == 38,
                                     skip_group_check=True)

                nc.sync.dma_start(
                    out=zu_in[:].rearrange("(mt p) c -> p mt c", p=128),
                    in_=zu_sb)
                if sim:
                    nc.sync.dma_start(out=zu_out[0:RPC, :], in_=zu_in[:])
                else:
                    nc.gpsimd.collective_compute(
                        "AllGather", OP.bypass, replica_groups=RG,
                        ins=[zu_in[:]], outs=[zu_out[:]])
                nc.sync.dma_start(
                    out=zuf,
                    in_=zu_out[:].rearrange("(kt p) c -> p kt c", p=128))

                # w' = adjC @ zu ; out = logsoftmax(0.5v - drow(0.5u'+w')+b2)
                for g in range(2):
                    mts = range(4 * g, 4 * g + 4)
                    psw = {mt: puw.tile([128, NC], F32, tag="pu",
                                        name=f"psw{mt}") for mt in mts}
                    for kt in range(KT):
                        for mt in mts:
                            nc.tensor.matmul(
                                psw[mt], adj_t[kt][:, ts(mt, 128)],
                                zuf[:, kt, :],
                                start=kt == 0, stop=kt == KT - 1)
                    G_ts, sm_ts = {}, {}
                    for mt in mts:
                        G_t = p_rot.tile([128, NC], F32, tag="G",
                                         name=f"G{mt}", bufs=4)
                        nc.vector.tensor_add(G_t, usb[:, mt, :], psw[mt])
                        nc.vector.tensor_scalar_mul(G_t, G_t,
                                                    ndr[:, mt:mt + 1])
                        nc.vector.tensor_add(G_t, G_t, vh_sb[:, mt, :])
                        nc.vector.tensor_add(G_t, G_t, b2_sb)
                        mx_t = p_rot.tile([128, 1], F32, tag="mx")
                        nc.vector.tensor_reduce(out=mx_t, in_=G_t,
                                                axis=AX.X, op=OP.max)
                        nc.vector.tensor_scalar(G_t, G_t, mx_t, None,
                                                op0=OP.subtract)
                        G_ts[mt] = G_t
                    for mt in mts:
                        ex_t = p_rot.tile([128, NC], F32, tag="ex")
                        sm_t = p_rot.tile([128, 1], F32, tag="sm",
                                          name=f"sm{mt}", bufs=4)
                        nc.scalar.activation(out=ex_t, in_=G_ts[mt],
                                             func=AF.Exp, accum_out=sm_t)
                        sm_ts[mt] = sm_t
                    for mt in mts:
                        lg_t = p_rot.tile([128, 1], F32, tag="lg")
                        nc.scalar.activation(out=lg_t, in_=sm_ts[mt],
                                             func=AF.Ln)
                        nc.vector.tensor_scalar(out_sb[:, mt, :], G_ts[mt],
                                                lg_t, None, op0=OP.subtract)
                nc.sync.dma_start(
                    out=out[:].rearrange("(mt p) c -> p mt c", p=128),
                    in_=out_sb)

    nc.compile()
    return nc


def _get_nc(lite=False):
    key = "nc_lite" if lite else "nc"
    if key not in _CACHE:
        _CACHE[key] = _build(lite=lite)
    return _CACHE[key]


def _prep_in_maps(x, adj, W1, W2, b2):
    bf = ml_dtypes.bfloat16
    f32 = np.float32
    x = np.asarray(x, f32)
    adj = np.asarray(adj, f32)
    w1 = np.asarray(W1, f32).astype(bf)
    w2h = (0.5 * np.asarray(W2, f32)).astype(bf)
    b2v = np.asarray(b2, f32).reshape(1, NC)
    in_maps = []
    for i in range(NCORE):
        rows = slice(i * RPC, (i + 1) * RPC)
        in_maps.append({
            "adjT": adj[rows, :].T.astype(bf),   # one fused copy+cast
            "xT": x[rows, :].T.astype(bf),
            "w1": w1, "w2h": w2h, "b2": b2v,
        })
    return in_maps


def _run(x, adj, W1, W2, b2, trace=False, lite=False, in_maps=None):
    from concourse.bass_utils import run_bass_kernel_spmd
    nc = _get_nc(lite=lite)
    if in_maps is None:
        in_maps = _prep_in_maps(x, adj, W1, W2, b2)
    res = run_bass_kernel_spmd(nc, in_maps, core_ids=list(range(NCORE)),
                               trace=trace)
    out = np.concatenate([r["out"] for r in res.results], axis=0)
    return out, res


def kernel(x, adj, W1, W2, b2):
    out, _ = _run(x, adj, W1, W2, b2, trace=False)
    return out
